# revision 25
# baseline (speedup 1.0000x reference)
"""Distributed Trainium2 kernel for the GNN message-passing model.

Self-contained: host-side structural prep (sharding, edge sort, index
remap) + Bass/Tile SPMD kernel across 8 NeuronCores.

Math (see reference):
  logits = MLP(x1); m = 0.15 + 0.55*onehot(argmax(logits))
  r1 = (m@W1s)*x2 + m@bp1
  g1 = relu(Dh A Dh (r1@gcn1_w) + gcn1_b); g1 = (m@W12)*g1 + 2e-4*(r1@W13)
  r2 = (m@W2s)*g1 + m@bp2
  g2 = relu(Dh A Dh (r2@gcn2_w) + gcn2_b)
  out = log_softmax(g2@fc_w + fc_b)
where Dh = diag(deg^-1/2), deg = in-degree over dst.

Distribution: nodes sharded contiguously over 8 cores. Per GCN layer the
scaled features h' = Dh*h are AllGathered in fp8 (in node chunks, so
comm overlaps the producer pipeline); each core gathers h'[src] for
edges whose dst it owns via indirect DMA and scatter-reduces them with
one-hot matmuls on the TensorEngine (PSUM accumulation per dst block).

fp8 scaling scheme (linear factors commute through relu/scatter):
  weights w1,w2,w3,g1w,W13,g2w scaled x16 host-side (avoids fp8
  subnormals); tables carry 64*dinv[src]*h; consumers unscale via the
  existing activation-scale slots (dinv/64).
Layer-2 scatter is transposed: psum[feat=32, dst=128] accumulates
lhsT=gathered values, rhs=one-hot masks; relu + fc run directly on the
transposed tile and dinv[dst]/64 is folded into the final psum->out
copy (per-partition scale), eliminating per-block transposes.
"""

import numpy as np

P = 128
TAU_HI = 0.7
TAU_LO = 0.15  # (1-0.7)/2
SW = 16.0     # weight upscale (w1,w2,w3,g1w,W13,g2w)
TS = 64.0     # gathered-table scale (both layers)


class _Cfg:
    def __init__(self, N, E, F1=768, H=512, G1=256, G2=32, FOUT=40, C=7):
        self.NC = 8
        self.N = N
        self.E = E
        self.NLOC_RAW = N // self.NC
        self.NB = -(-self.NLOC_RAW // P)          # node blocks per core
        self.NLOC = self.NB * P
        assert self.NB % C == 0, (self.NB, C)
        self.C = C                                 # allgather chunks
        self.BPC = self.NB // C                    # blocks per chunk
        self.CH = self.BPC * P                     # chunk nodes
        self.TR = self.NC * self.NLOC              # gathered table rows
        self.CHR = self.NC * self.CH               # rows per chunk in table
        self.F1, self.H, self.G1, self.G2, self.FOUT = F1, H, G1, G2, FOUT
        self.KF1 = F1 // P                         # 6 k-tiles
        self.KH = H // P                           # 4
        self.KG1 = G1 // P                         # 2
        self.RB1 = [0, 3, 6, C]                    # L1 round chunk bounds
        self.SPL2 = max(1, C - 3)                  # L2 round-A src chunks
        self.NFREE = min(448, self.CH)             # front free-dim unit
        assert self.CH % self.NFREE == 0
        self.FU = self.CH // self.NFREE            # free units per chunk


CFG_FULL = dict(N=50000, E=800000)


def _to_bf16(x):
    import ml_dtypes
    return np.asarray(x, np.float32).astype(ml_dtypes.bfloat16)


def _to_f8(x):
    import ml_dtypes
    return np.asarray(x, np.float32).astype(ml_dtypes.float8_e4m3)


def _row_of_node(v, cfg):
    """Gathered-table row for global node id v (vectorized).

    Layout within a (chunk k, rank c) slice is partition-major
    (row = p*BPC + nb) so the producer can scatter a whole chunk with a
    single base+consecutive-rows indirect DMA."""
    c = v // cfg.NLOC_RAW
    s = v - c * cfg.NLOC_RAW
    k = s // cfg.CH
    off = s - k * cfg.CH
    nb = off // P
    p = off - nb * P
    return k * cfg.CHR + c * cfg.CH + p * cfg.BPC + nb


def host_prep(inputs, cfg):
    """Returns (in_maps, sched). sched is baked into the built graph and
    must be identical for every core (SPMD)."""
    x1 = np.asarray(inputs["x1"], np.float32)
    x2 = np.asarray(inputs["x2"], np.float32)
    ei = np.asarray(inputs["edge_index"])
    src = ei[0].astype(np.int64)
    dst = ei[1].astype(np.int64)
    N, E, NC = cfg.N, cfg.E, cfg.NC
    assert x1.shape[0] == N and src.shape[0] == E

    deg = np.bincount(dst, minlength=N).astype(np.float64)
    dinv = np.where(deg > 0, deg ** -0.5, 0.0).astype(np.float32)
    sdeg = np.sqrt(deg).astype(np.float32)  # 1/dinv where deg>0 else 0

    # ---- per-core edge partition by dst owner, sorted by dst block ----
    owner = dst // cfg.NLOC_RAW
    dloc = dst - owner * cfg.NLOC_RAW
    dblk = dloc // P
    drel_all = (dloc - dblk * P).astype(np.float32)
    rows_all = _row_of_node(src, cfg).astype(np.int32)

    per_core = []
    cnt = np.zeros((NC, cfg.NB), np.int64)
    for c in range(NC):
        sel = np.where(owner == c)[0]
        order = np.argsort(dblk[sel], kind="stable")
        sel = sel[order]
        b_of = dblk[sel]
        bounds = np.searchsorted(b_of, np.arange(cfg.NB + 1))
        lists = []
        for b in range(cfg.NB):
            idxs = sel[bounds[b]:bounds[b + 1]]
            lists.append((rows_all[idxs], drel_all[idxs]))
            cnt[c, b] = len(idxs)
        per_core.append(lists)

    # Two uniform cross-core layouts, each split in 2 rounds by src chunk
    # (round boundary = which AllGather chunks the gathers depend on).
    def build_layout(chunk_bounds, pad_mult):
        bounds_k = [b * cfg.CHR for b in chunk_bounds]
        layout = dict(rounds=[])
        for r in range(len(chunk_bounds) - 1):
            lo, hi = bounds_k[r], bounds_k[r + 1]
            cntr = np.zeros((NC, cfg.NB), np.int64)
            per_rc = []
            for c in range(NC):
                pc = []
                for b in range(cfg.NB):
                    rows, rel = per_core[c][b]
                    m = (rows >= lo) & (rows < hi)
                    pc.append((rows[m] - lo, rel[m]))
                    cntr[c, b] = int(m.sum())
                per_rc.append(pc)
            Kb = np.maximum(1, -(-cntr.max(axis=0) // P)).astype(np.int64)
            nb_round = int(Kb.sum())
            pad = (-nb_round) % pad_mult
            nb_round += pad
            b_of = np.concatenate([np.repeat(np.arange(cfg.NB), Kb),
                                   np.full(pad, cfg.NB - 1)])
            first = np.zeros(nb_round, bool)
            last = np.zeros(nb_round, bool)
            skip = np.ones(nb_round, bool)
            off = 0
            for b in range(cfg.NB):
                first[off] = True
                e = off + int(Kb[b])
                if b == cfg.NB - 1:
                    e = nb_round
                last[e - 1] = True
                for jj in range(int(Kb[b])):
                    skip[off + jj] = not bool((cntr[:, b] > jj * P).any())
                off += int(Kb[b])
            layout["rounds"].append(dict(Kb=Kb, nblocks=nb_round, b_of=b_of,
                                         first=first, last=last, skip=skip,
                                         per_rc=per_rc))
        return layout

    lay1 = build_layout(cfg.RB1, 16)
    lay2 = build_layout([0, cfg.SPL2, cfg.C], 32)

    def pack_layout(layout, c, JW):
        idxs, Ss = [], []
        for rr in layout["rounds"]:
            sbs = rr["nblocks"] // 8
            idx = np.zeros((sbs * P, 8), np.int32)
            drl = np.full((sbs * P, 8), -1.0, np.float32)
            g = 0
            for b in range(cfg.NB):
                rows, rel = rr["per_rc"][c][b]
                n = len(rows)
                nblk = int(rr["Kb"][b])
                if b == cfg.NB - 1:
                    nblk = rr["nblocks"] - g
                for j in range(nblk):
                    s, jj = g // 8, g % 8
                    e0 = j * P
                    m = min(P, max(0, n - e0))
                    if m > 0:
                        idx[s * P:s * P + m, jj] = rows[e0:e0 + m]
                        drl[s * P:s * P + m, jj] = rel[e0:e0 + m]
                    g += 1
            # regroup [sbs*P, 8] -> [(nblocks//JW)*P, JW] so device loads
            # are plain 2D slices (per-iteration rows)
            g8 = JW // 8
            nq = sbs // g8
            idx = (idx.reshape(nq, g8, P, 8).transpose(0, 2, 1, 3)
                   .reshape(nq * P, JW))
            drl = (drl.reshape(nq, g8, P, 8).transpose(0, 2, 1, 3)
                   .reshape(nq * P, JW))
            idxs.append(idx.copy())
            Ss.append(_to_bf16(drl))
        return (np.concatenate(idxs, axis=0), np.concatenate(Ss, axis=0))

    def mk(lay):
        return [dict(nblocks=r["nblocks"], b_of=r["b_of"], first=r["first"],
                     last=r["last"], skip=r["skip"]) for r in lay["rounds"]]
    sched = dict(lay1=mk(lay1), lay2=mk(lay2))

    # ---- weights ----
    w1 = np.asarray(inputs["mlp_w1"], np.float32)
    w2 = np.asarray(inputs["mlp_w2"], np.float32)
    w3 = np.asarray(inputs["mlp_w3"], np.float32)
    b1 = np.asarray(inputs["mlp_b1"], np.float32)
    b2 = np.asarray(inputs["mlp_b2"], np.float32)
    b3 = np.asarray(inputs["mlp_b3"], np.float32)
    W1s = np.asarray(inputs["W1"], np.float32).sum(-1)
    W12 = np.asarray(inputs["W12"], np.float32)
    W13 = np.asarray(inputs["W13"], np.float32)  # 2e-4 folded on-device
    bp1 = np.asarray(inputs["bp1"], np.float32)
    W2s = np.asarray(inputs["W2"], np.float32).sum(-1)
    bp2 = np.asarray(inputs["bp2"], np.float32)
    g1w = np.asarray(inputs["gcn1_w"], np.float32)
    g1b = np.asarray(inputs["gcn1_b"], np.float32)
    g2w = np.asarray(inputs["gcn2_w"], np.float32)
    g2b = np.asarray(inputs["gcn2_b"], np.float32)
    fcw = np.asarray(inputs["fc_w"], np.float32)
    fcb = np.asarray(inputs["fc_b"], np.float32)

    sched["bp1_nz"] = bool(np.any(bp1 != 0))
    sched["bp2_nz"] = bool(np.any(bp2 != 0))
    sched["g1b_nz"] = bool(np.any(g1b != 0))
    sched["g2b_nz"] = bool(np.any(g2b != 0))
    sched["fcb_nz"] = bool(np.any(fcb != 0))
    sched["b3_nz"] = bool(np.any(b3 != 0))
    sched["w12_ones"] = bool(np.all(W12 == 1.0))

    def pack_lhsT(w, KT, MT):
        o = np.zeros((P, KT * MT * P), np.float32)
        for k in range(KT):
            for m in range(MT):
                o[:, (k * MT + m) * P:(k * MT + m + 1) * P] = \
                    w[k * P:(k + 1) * P, m * P:(m + 1) * P]
        return o

    def pack_rhs(w, KT, F):
        o = np.zeros((P, KT * F), np.float32)
        for k in range(KT):
            o[:, k * F:(k + 1) * F] = w[k * P:(k + 1) * P, :]
        return o

    def pack_k3(w, F):
        o = np.zeros((4, F), np.float32)
        o[:3] = w
        return _to_bf16(o)

    w1_p = _to_f8(pack_lhsT(w1 * SW, cfg.KF1, cfg.KH))
    w2_p = _to_f8(pack_lhsT(w2 * SW, cfg.KH, cfg.KH))
    w3_p = _to_f8(pack_rhs(np.pad(w3 * SW, ((0, 0), (0, 1))), cfg.KH, 4))
    b1_p = b1.reshape(cfg.KH, P).T.copy()
    b2_p = b2.reshape(cfg.KH, P).T.copy()
    b3_p = np.pad(b3 * SW, (0, 1)).reshape(1, 4).repeat(P, 0).copy()
    # fused gcn1 rhs: per k-slab [g1w*16 | W13*16] -> [P, KF1*2*G1]
    gw = np.zeros((P, cfg.KF1 * 2 * cfg.G1), np.float32)
    for k in range(cfg.KF1):
        gw[:, k * 2 * cfg.G1:k * 2 * cfg.G1 + cfg.G1] = \
            g1w[k * P:(k + 1) * P, :] * SW
        gw[:, k * 2 * cfg.G1 + cfg.G1:(k + 1) * 2 * cfg.G1] = \
            W13[k * P:(k + 1) * P, :] * SW
    gw_p = _to_f8(gw)
    g2w_p = _to_bf16(pack_rhs(g2w * SW, cfg.KG1, cfg.G2))
    fcw_p = _to_bf16(fcw)
    W1s_p = pack_k3(W1s, cfg.F1)
    bp1_p = pack_k3(bp1, cfg.F1)
    W12_p = pack_k3(W12, cfg.G1)
    W2s_p = pack_k3(W2s, cfg.G1)
    bp2_p = pack_k3(bp2, cfg.G1)
    g1b_p = _to_bf16(g1b.reshape(1, cfg.G1))
    g2b_p = _to_bf16(g2b.reshape(1, cfg.G2))
    fcb_p = np.repeat(fcb.reshape(1, cfg.FOUT), P, axis=0).astype(np.float32)

    in_maps = []
    for c in range(NC):
        lo = c * cfg.NLOC_RAW
        hi = lo + cfg.NLOC_RAW
        x1T = np.zeros((cfg.F1, cfg.NLOC), np.float32)
        x1T[:, :cfg.NLOC_RAW] = x1[lo:hi].T
        x2T = np.zeros((cfg.F1, cfg.NLOC), np.float32)
        x2T[:, :cfg.NLOC_RAW] = x2[lo:hi].T

        def dpack(v):
            t = np.zeros(cfg.NLOC, np.float32)
            t[:cfg.NLOC_RAW] = v[lo:hi]
            return t.reshape(cfg.NB, P).T.copy()

        dinv4_t = dpack(dinv * (TS / SW))
        dinv16_t = dpack(dinv * SW)
        dlo_t = dpack(dinv / TS)
        sdeg_r = np.zeros((1, cfg.NLOC), np.float32)
        sdeg_r[0, :cfg.NLOC_RAW] = sdeg[lo:hi] * TS

        idx1, drel1 = pack_layout(lay1, c, 16)
        idx2, drel2 = pack_layout(lay2, c, 32)
        rows1 = (c * cfg.CH
                 + np.arange(P, dtype=np.int32)[:, None] * cfg.BPC
                 ).astype(np.int32)
        iota = np.tile(np.arange(P, dtype=np.float32), 32).reshape(1, 32 * P)
        im = {
            "identb": _to_bf16(np.eye(P, dtype=np.float32)),
            "identf": _to_f8(np.eye(P, dtype=np.float32)),
            "iota": _to_bf16(np.repeat(iota, P, axis=0)),
            "x1T": _to_f8(x1T), "x2T": _to_bf16(x2T),
            "idx1": idx1, "drel1": drel1, "idx2": idx2, "drel2": drel2,
            "rows1": rows1,
            "dinv4": dinv4_t, "dinv16": dinv16_t, "dlo": dlo_t,
            "sdeg64": _to_bf16(sdeg_r),
            "w1": w1_p, "w2": w2_p, "w3": w3_p,
            "b1": b1_p, "b2": b2_p, "b3": b3_p,
            "gw": gw_p, "g2w": g2w_p, "fcw": fcw_p,
            "W1s": W1s_p, "bp1": bp1_p, "W12": W12_p, "W2s": W2s_p,
            "bp2": bp2_p, "g1b": g1b_p, "g2b": g2b_p, "fcb": fcb_p,
        }
        in_maps.append(im)
    return in_maps, sched


def build(cfg, sched, debug=False, debug_dump=False):
    import concourse.bacc as bacc
    import concourse.bass as bass
    import concourse.mybir as mybir
    import concourse.tile as tile

    dt = mybir.dt
    AF = mybir.ActivationFunctionType
    OP = mybir.AluOpType
    AX = mybir.AxisListType

    nc = bacc.Bacc("TRN2", target_bir_lowering=False, debug=debug)

    NB, C, BPC, CH, NLOC, TR, CHR = (cfg.NB, cfg.C, cfg.BPC, cfg.CH,
                                     cfg.NLOC, cfg.TR, cfg.CHR)
    F1, H, G1, G2, FOUT = cfg.F1, cfg.H, cfg.G1, cfg.G2, cfg.FOUT
    KF1, KH, KG1 = cfg.KF1, cfg.KH, cfg.KG1
    NF, FU = cfg.NFREE, cfg.FU
    L1R = sched["lay1"]
    L2A, L2B = sched["lay2"]
    SB1 = sum(r["nblocks"] for r in L1R) // 8
    SB2T = (L2A["nblocks"] + L2B["nblocks"]) // 8
    RB1, SPL2 = cfg.RB1, cfg.SPL2
    NR1 = len(RB1) - 1

    bf = dt.bfloat16
    f8 = dt.float8e4
    f32 = dt.float32

    dd = {}

    def din(name, shape, dtype):
        dd[name] = nc.declare_dram_parameter(name, list(shape), dtype,
                                             isOutput=False)
        return dd[name]

    x1T_d = din("x1T", [F1, NLOC], f8)
    x2T_d = din("x2T", [F1, NLOC], bf)
    idx1_d = din("idx1", [SB1 // 2 * P, 16], dt.int32)
    drel1_d = din("drel1", [SB1 // 2 * P, 16], bf)
    idx2_d = din("idx2", [SB2T // 4 * P, 32], dt.int32)
    drel2_d = din("drel2", [SB2T // 4 * P, 32], bf)
    iota_d = din("iota", [P, 32 * P], bf)
    rows1_d = din("rows1", [P, 1], dt.int32)
    dinv4_d = din("dinv4", [P, NB], f32)
    dinv16_d = din("dinv16", [P, NB], f32)
    dlo_d = din("dlo", [P, NB], f32)
    sdeg_d = din("sdeg64", [1, NLOC], bf)
    w1_d = din("w1", [P, KF1 * KH * P], f8)
    w2_d = din("w2", [P, KH * KH * P], f8)
    w3_d = din("w3", [P, KH * 4], f8)
    b1_d = din("b1", [P, KH], f32)
    b2_d = din("b2", [P, KH], f32)
    b3_d = din("b3", [P, 4], f32)
    gw_d = din("gw", [P, KF1 * 2 * G1], f8)
    g2w_d = din("g2w", [P, KG1 * G2], bf)
    fcw_d = din("fcw", [G2, FOUT], bf)
    W1s_d = din("W1s", [4, F1], bf)
    bp1_d = din("bp1", [4, F1], bf)
    W12_d = din("W12", [4, G1], bf)
    W2s_d = din("W2s", [4, G1], bf)
    bp2_d = din("bp2", [4, G1], bf)
    g1b_d = din("g1b", [1, G1], bf)
    g2b_d = din("g2b", [1, G2], bf)
    fcb_d = din("fcb", [P, FOUT], f32)
    identb_d = din("identb", [P, P], bf)
    identf_d = din("identf", [P, P], f8)
    out_d = nc.declare_dram_parameter("out", [NLOC, FOUT], f32, isOutput=True)
    if debug_dump:
        dbg_gath = nc.declare_dram_parameter("dbg_gath", [P, 64 * G2], f8,
                                             isOutput=True)
        dbg_t1 = nc.declare_dram_parameter("dbg_t1", [4 * P, G1], f8,
                                           isOutput=True)
        dbg_z = nc.declare_dram_parameter("dbg_z", [P, NB * G1], bf,
                                          isOutput=True)
        dbg_sc = nc.declare_dram_parameter("dbg_sc", [P, 4 * NB], f32,
                                           isOutput=True)
        dbg_stg = nc.declare_dram_parameter("dbg_stg", [P, BPC * G1], f8,
                                            isOutput=True)
        dbg_g1a = nc.declare_dram_parameter("dbg_g1a", [4 * P, G1], f8,
                                            isOutput=True)
        dbg_gidx = nc.declare_dram_parameter("dbg_gidx", [P, 64], dt.int32,
                                             isOutput=False)
        dbg_agg2 = nc.declare_dram_parameter("dbg_agg2", [G2, NB * P], bf,
                                             isOutput=True)
        dbg_mt = nc.declare_dram_parameter("dbg_mt", [4, NLOC], bf,
                                           isOutput=True)

    with tile.TileContext(nc) as tc:
        with (
            tc.tile_pool(name="const", bufs=1) as cp,
            tc.tile_pool(name="front", bufs=2) as fp,
            tc.tile_pool(name="scat", bufs=3) as sp,
            tc.tile_pool(name="fin", bufs=2) as qp,
            tc.tile_pool(name="psG", bufs=2, space="PSUM") as psG,
            tc.tile_pool(name="psS", bufs=2, space="PSUM") as psS,
            tc.tile_pool(name="psW", bufs=2, space="PSUM") as psW,
            tc.tile_pool(name="psT", bufs=2, space="PSUM") as psT,
            tc.tile_pool(name="dram", bufs=1, space="DRAM") as dp,
        ):
            def load(dr, shape, dtype, name):
                t = cp.tile(shape, dtype, tag=name)
                nc.sync.dma_start(out=t[:, :], in_=dr[:, :])
                return t

            w1_s = load(w1_d, [P, KF1 * KH * P], f8, "w1")
            w2_s = load(w2_d, [P, KH * KH * P], f8, "w2")
            w3_s = load(w3_d, [P, KH * 4], f8, "w3")
            b1_s = load(b1_d, [P, KH], f32, "b1")
            b2_s = load(b2_d, [P, KH], f32, "b2")
            b3_s = load(b3_d, [P, 4], f32, "b3")
            gw_s = load(gw_d, [P, KF1 * 2 * G1], f8, "gw")
            g2w_s = load(g2w_d, [P, KG1 * G2], bf, "g2w")
            fcw_s = load(fcw_d, [G2, FOUT], bf, "fcw")
            W1s_s = load(W1s_d, [4, F1], bf, "W1s")
            bp1_s = load(bp1_d, [4, F1], bf, "bp1")
            W12_s = load(W12_d, [4, G1], bf, "W12")
            W2s_s = load(W2s_d, [4, G1], bf, "W2s")
            bp2_s = load(bp2_d, [4, G1], bf, "bp2")
            g1b_s = load(g1b_d, [1, G1], bf, "g1b")
            g2b_s = load(g2b_d, [1, G2], bf, "g2b")
            fcb_s = load(fcb_d, [P, FOUT], f32, "fcb")
            dinv4_s = load(dinv4_d, [P, NB], f32, "dinv4")
            dinv16_s = load(dinv16_d, [P, NB], f32, "dinv16")
            dlo_s = load(dlo_d, [P, NB], f32, "dlo")
            sdeg_s = load(sdeg_d, [1, NLOC], bf, "sdeg")

            identb = load(identb_d, [P, P], bf, "identb")
            identf = load(identf_d, [P, P], f8, "identf")
            iota_s = load(iota_d, [P, 32 * P], bf, "iota")
            rows1_s = load(rows1_d, [P, 1], dt.int32, "rows1")
            ztile = cp.tile([P, 14 * G1], f8, tag="ztile")
            nc.vector.memset(ztile[:, :], 0.0)

            mT_s = cp.tile([4, NLOC], bf, tag="mT")
            out_acc = cp.tile([P, NB * FOUT], f32, tag="oacc")
            z_s = cp.tile([P, NB * G1], bf, tag="z")
            aggA_s = cp.tile([P, NB * G1], bf, tag="aggA")
            agg2_s = cp.tile([G2, NB * P], bf, tag="agg2")

            h1t = [dp.tile([CHR, G1], f8, tag=f"h1t{k}", name=f"h1t{k}")
                   for k in range(C)]
            h2t = [dp.tile([CHR, G2], f8, tag=f"h2t{k}", name=f"h2t{k}")
                   for k in range(C)]
            h2stg = cp.tile([P, NB * G2], f8, tag="h2stg")
            # zero-fill AR input tables upfront (remote slots must be 0;
            # AllReduce(add) over disjoint slots emulates AllGather, which
            # has broken semantics in this runtime)
            ZB = CHR // P // 4
            for k in range(C):
                for z4 in range(4):
                    nc.sync.dma_start(
                        out=h1t[k][z4 * ZB * P:(z4 + 1) * ZB * P, :]
                            .rearrange("(a p) e -> p a e", p=P),
                        in_=ztile[:, :ZB * G1]
                            .rearrange("p (a e) -> p a e", e=G1))
            for k in range(C):
                for z4 in range(4):
                    nc.sync.dma_start(
                        out=h2t[k][z4 * ZB * P:(z4 + 1) * ZB * P, :]
                            .rearrange("(a p) e -> p a e", p=P),
                        in_=ztile[:, :ZB * G2]
                            .rearrange("p (a e) -> p a e", e=G2))
            h1g = [dp.tile([(RB1[r + 1] - RB1[r]) * CHR, G1], f8,
                           tag=f"h1g{r}", name=f"h1g{r}")
                   for r in range(NR1)]
            h2gA = dp.tile([SPL2 * CHR, G2], f8, tag="h2gA")
            h2gB = dp.tile([(C - SPL2) * CHR, G2], f8, tag="h2gB")

            # ================= FRONT (per chunk) =================
            for k in range(C):
                n0 = k * CH
                x1c = fp.tile([P, KF1 * CH], f8, tag="x1c")
                nc.sync.dma_start(
                    out=x1c[:, :].rearrange("p (a n) -> p a n", n=CH),
                    in_=x1T_d[:, n0:n0 + CH].rearrange("(a p) n -> p a n", p=P))
                x2c = fp.tile([P, KF1 * CH], bf, tag="x2c", bufs=1)
                nc.sync.dma_start(
                    out=x2c[:, :].rearrange("p (a n) -> p a n", n=CH),
                    in_=x2T_d[:, n0:n0 + CH].rearrange("(a p) n -> p a n", p=P))

                h1T = fp.tile([P, KH * CH], f8, tag="h1T", bufs=1)
                for u in range(FU):
                    for m in range(KH):
                        ps = psG.tile([P, NF], f32, tag="g")
                        for kk in range(KF1):
                            nc.tensor.matmul(
                                ps[:, :],
                                lhsT=w1_s[:, (kk * KH + m) * P:(kk * KH + m + 1) * P],
                                rhs=x1c[:, kk * CH + u * NF:kk * CH + u * NF + NF],
                                start=(kk == 0), stop=(kk == KF1 - 1))
                        nc.scalar.activation(
                            h1T[:, m * CH + u * NF:m * CH + u * NF + NF],
                            ps[:, :], AF.Relu, bias=b1_s[:, m:m + 1],
                            scale=1.0 / SW)
                h2T = fp.tile([P, KH * CH], f8, tag="h2T", bufs=1)
                for u in range(FU):
                    for m in range(KH):
                        ps = psG.tile([P, NF], f32, tag="g")
                        for kk in range(KH):
                            nc.tensor.matmul(
                                ps[:, :],
                                lhsT=w2_s[:, (kk * KH + m) * P:(kk * KH + m + 1) * P],
                                rhs=h1T[:, kk * CH + u * NF:kk * CH + u * NF + NF],
                                start=(kk == 0), stop=(kk == KH - 1))
                        nc.scalar.activation(
                            h2T[:, m * CH + u * NF:m * CH + u * NF + NF],
                            ps[:, :], AF.Relu, bias=b2_s[:, m:m + 1],
                            scale=1.0 / SW)

                mmc = fp.tile([P, BPC * 3], bf, tag="mmc")
                for nb in range(BPC):
                    psl = psW.tile([P, 512], f32, tag="w")
                    for kk in range(KH):
                        nc.tensor.matmul(
                            psl[:, :4],
                            lhsT=h2T[:, kk * CH + nb * P:kk * CH + (nb + 1) * P],
                            rhs=w3_s[:, kk * 4:(kk + 1) * 4],
                            start=(kk == 0), stop=(kk == KH - 1))
                    lg = fp.tile([P, 3], f32, tag="lg")
                    if sched["b3_nz"]:
                        nc.vector.tensor_add(lg[:, :], psl[:, :3], b3_s[:, :3])
                    else:
                        nc.vector.tensor_copy(lg[:, :], psl[:, :3])
                    rmax = fp.tile([P, 1], f32, tag="rmax")
                    nc.vector.reduce_max(rmax[:, :], lg[:, :], axis=AX.X)
                    mm = fp.tile([P, 3], bf, tag="mm")
                    nc.vector.tensor_scalar(
                        mm[:, :], lg[:, :], rmax[:, :1], None, OP.is_equal)
                    nc.scalar.activation(mmc[:, nb * 3:(nb + 1) * 3],
                                         mm[:, :], AF.Copy,
                                         bias=TAU_LO, scale=TAU_HI - TAU_LO)
                for nb in range(BPC):
                    b_glob = k * BPC + nb
                    pst = psT.tile([P, P], bf, tag="t")
                    nc.tensor.transpose(pst[:3, :],
                                        mmc[:, nb * 3:(nb + 1) * 3],
                                        identb[:, :])
                    nc.vector.tensor_copy(
                        mT_s[:3, b_glob * P:(b_glob + 1) * P], pst[:3, :])

                r1T = fp.tile([P, KF1 * CH], f8, tag="r1T")
                for u in range(FU):
                    for f in range(KF1):
                        psr = psG.tile([P, NF], f32, tag="g")
                        nc.tensor.matmul(
                            psr[:, :], lhsT=W1s_s[:3, f * P:(f + 1) * P],
                            rhs=mT_s[:3, n0 + u * NF:n0 + u * NF + NF],
                            start=True, stop=True)
                        if sched["bp1_nz"]:
                            psr2 = psW.tile([P, 512], f32, tag="w")
                            nc.tensor.matmul(
                                psr2[:, :NF], lhsT=bp1_s[:3, f * P:(f + 1) * P],
                                rhs=mT_s[:3, n0 + u * NF:n0 + u * NF + NF],
                                start=True, stop=True)
                            tmp = fp.tile([P, NF], f32, tag="r1tmp")
                            nc.vector.tensor_mul(
                                tmp[:, :], psr[:, :],
                                x2c[:, f * CH + u * NF:f * CH + u * NF + NF])
                            nc.vector.tensor_add(
                                r1T[:, f * CH + u * NF:f * CH + u * NF + NF],
                                tmp[:, :], psr2[:, :NF])
                        else:
                            nc.vector.tensor_mul(
                                r1T[:, f * CH + u * NF:f * CH + u * NF + NF],
                                psr[:, :],
                                x2c[:, f * CH + u * NF:f * CH + u * NF + NF])

                h1stg = fp.tile([P, BPC * G1], f8, tag="h1stg")
                for nb in range(BPC):
                    b_glob = k * BPC + nb
                    psh = psW.tile([P, 512], f32, tag="w")
                    for f in range(KF1):
                        nc.tensor.matmul(
                            psh[:, :],
                            lhsT=r1T[:, f * CH + nb * P:f * CH + (nb + 1) * P],
                            rhs=gw_s[:, f * 2 * G1:(f + 1) * 2 * G1],
                            start=(f == 0), stop=(f == KF1 - 1))
                    nc.scalar.activation(h1stg[:, nb * G1:(nb + 1) * G1],
                                         psh[:, :G1], AF.Copy,
                                         scale=dinv4_s[:, b_glob:b_glob + 1])
                    nc.scalar.activation(
                        z_s[:, b_glob * G1:(b_glob + 1) * G1],
                        psh[:, G1:2 * G1], AF.Copy, scale=2e-4 / SW)

                if debug_dump and k == C - 1:
                    nc.sync.dma_start(out=dbg_stg[:, :], in_=h1stg[:, :])
                nc.gpsimd.indirect_dma_start(
                    out=h1t[k][:, :],
                    out_offset=bass.IndirectOffsetOnAxis(
                        ap=rows1_s[:, :], axis=0),
                    in_=h1stg[:, :], in_offset=None)
                r1r = next(r for r in range(NR1)
                           if RB1[r] <= k < RB1[r + 1])
                kk0 = k - RB1[r1r]
                agt = h1g[r1r][kk0 * CHR:(kk0 + 1) * CHR, :]
                nc.gpsimd.collective_compute(
                    "AllReduce", OP.add,
                    replica_groups=[list(range(cfg.NC))],
                    ins=[h1t[k][:, :].opt()],
                    outs=[agt.opt()])

            # ================= LAYER 1 scatter (2 rounds) =================
            ps_by_b = {}

            def l1_finalize(b):
                psb = ps_by_b.pop(b)
                if sched["g1b_nz"]:
                    nc.tensor.matmul(
                        psb[:, :], lhsT=sdeg_s[:1, b * P:(b + 1) * P],
                        rhs=g1b_s[:1, :], start=False, stop=True,
                        skip_group_check=True)
                g1r = qp.tile([P, G1], bf, tag="g1r", bufs=3)
                nc.scalar.activation(g1r[:, :], psb[:, :], AF.Relu,
                                     scale=dlo_s[:, b:b + 1])
                psmw = psW.tile([P, 512], f32, tag="w")
                if not sched["w12_ones"]:
                    nc.tensor.matmul(psmw[:, :G1],
                                     lhsT=mT_s[:3, b * P:(b + 1) * P],
                                     rhs=W12_s[:3, :], start=True, stop=True)
                nc.tensor.matmul(psmw[:, G1:2 * G1],
                                 lhsT=mT_s[:3, b * P:(b + 1) * P],
                                 rhs=W2s_s[:3, :], start=True, stop=True)
                g1v = qp.tile([P, G1], bf, tag="g1v", bufs=3)
                if sched["w12_ones"]:
                    nc.vector.tensor_add(g1v[:, :], g1r[:, :],
                                         z_s[:, b * G1:(b + 1) * G1])
                else:
                    g1t = qp.tile([P, G1], bf, tag="g1t", bufs=3)
                    nc.vector.tensor_mul(g1t[:, :], g1r[:, :], psmw[:, :G1])
                    nc.vector.tensor_add(g1v[:, :], g1t[:, :],
                                         z_s[:, b * G1:(b + 1) * G1])
                r2 = qp.tile([P, G1], bf, tag="r2", bufs=3)
                if sched["bp2_nz"]:
                    psm3 = psW.tile([P, 512], f32, tag="w")
                    nc.tensor.matmul(psm3[:, :G1],
                                     lhsT=mT_s[:3, b * P:(b + 1) * P],
                                     rhs=bp2_s[:3, :], start=True, stop=True)
                    r2u = qp.tile([P, G1], f32, tag="r2u")
                    nc.vector.tensor_mul(r2u[:, :], g1v[:, :],
                                         psmw[:, G1:2 * G1])
                    r2v = qp.tile([P, G1], f32, tag="r2v")
                    nc.vector.tensor_add(r2v[:, :], r2u[:, :], psm3[:, :G1])
                    nc.vector.tensor_scalar(r2[:, :], r2v[:, :],
                                            dinv16_s[:, b:b + 1], None,
                                            OP.mult)
                else:
                    nc.vector.scalar_tensor_tensor(
                        out=r2[:, :], in0=g1v[:, :],
                        scalar=dinv16_s[:, b:b + 1],
                        in1=psmw[:, G1:2 * G1], op0=OP.mult, op1=OP.mult)
                r2T = qp.tile([P, KG1 * P], bf, tag="r2T", bufs=3)
                for f in range(KG1):
                    pst = psT.tile([P, P], bf, tag="t")
                    nc.tensor.transpose(pst[:, :], r2[:, f * P:(f + 1) * P],
                                        identb[:, :])
                    nc.vector.tensor_copy(r2T[:, f * P:(f + 1) * P],
                                          pst[:, :])
                psh2 = psW.tile([P, 512], f32, tag="w")
                for f in range(KG1):
                    nc.tensor.matmul(
                        psh2[:, :G2], lhsT=r2T[:, f * P:(f + 1) * P],
                        rhs=g2w_s[:, f * G2:(f + 1) * G2],
                        start=(f == 0), stop=(f == KG1 - 1))
                nc.scalar.activation(h2stg[:, b * G2:(b + 1) * G2],
                                     psh2[:, :G2], AF.Copy,
                                     scale=1.0 / 4.0)
                k, nb = b // BPC, b % BPC
                if nb == BPC - 1:
                    nc.gpsimd.indirect_dma_start(
                        out=h2t[k][:, :],
                        out_offset=bass.IndirectOffsetOnAxis(
                            ap=rows1_s[:, :], axis=0),
                        in_=h2stg[:, k * BPC * G2:(k + 1) * BPC * G2],
                        in_offset=None)
                    agt2 = (h2gA[k * CHR:(k + 1) * CHR, :] if k < SPL2 else
                            h2gB[(k - SPL2) * CHR:(k - SPL2 + 1) * CHR, :])
                    nc.gpsimd.collective_compute(
                        "AllReduce", OP.add,
                        replica_groups=[list(range(cfg.NC))],
                        ins=[h2t[k][:, :].opt()], outs=[agt2.opt()])

            def l1_round(meta, sb_base16, table, is_first, is_last):
                for s_loc in range(meta["nblocks"] // 16):
                    it = sb_base16 + s_loc
                    gt = sp.tile([P, 16 * G1], f8, tag="gt1", bufs=2)
                    ix = sp.tile([P, 16], dt.int32, tag="ix1")
                    nc.sync.dma_start(out=ix[:, :],
                                      in_=idx1_d[it * P:(it + 1) * P, :])
                    for jg in range(16):
                        if meta["skip"][s_loc * 16 + jg]:
                            continue
                        nc.gpsimd.indirect_dma_start(
                            out=gt[:, jg * G1:(jg + 1) * G1],
                            out_offset=None, in_=table[:, :],
                            in_offset=bass.IndirectOffsetOnAxis(
                                ap=ix[:, jg:jg + 1], axis=0))
                    dr = sp.tile([P, 16], bf, tag="dr1")
                    nc.sync.dma_start(out=dr[:, :],
                                      in_=drel1_d[it * P:(it + 1) * P, :])
                    Ssb = sp.tile([P, 16 * P], f8, tag="S1", bufs=2)
                    nc.vector.tensor_tensor(
                        out=Ssb[:, :].rearrange("p (j c) -> p j c", c=P),
                        in0=iota_s[:, :16 * P].rearrange("p (j c) -> p j c",
                                                         c=P),
                        in1=dr[:, :].unsqueeze(2).to_broadcast([P, 16, P]),
                        op=OP.is_equal)
                    for j in range(16):
                        g = s_loc * 16 + j
                        b = int(meta["b_of"][g])
                        first = bool(meta["first"][g])
                        last = bool(meta["last"][g])
                        if first:
                            psb = psS.tile([P, G1], f32, tag="agg",
                                           name="agg1")
                            ps_by_b[b] = psb
                            if not is_first:
                                nc.tensor.matmul(
                                    psb[:, :], lhsT=identb[:, :],
                                    rhs=aggA_s[:, b * G1:(b + 1) * G1],
                                    start=True, stop=False)
                        psb = ps_by_b[b]
                        stop = last and (not sched["g1b_nz"]
                                         if is_last else True)
                        nc.tensor.matmul(
                            psb[:, :], lhsT=Ssb[:, j * P:(j + 1) * P],
                            rhs=gt[:, j * G1:(j + 1) * G1],
                            start=(first and is_first), stop=stop)
                        if not last:
                            continue
                        if not is_last:
                            nc.vector.tensor_copy(
                                aggA_s[:, b * G1:(b + 1) * G1],
                                ps_by_b.pop(b)[:, :])
                        else:
                            l1_finalize(b)

            sb16 = 0
            for r in range(NR1):
                l1_round(L1R[r], sb16, h1g[r], r == 0, r == NR1 - 1)
                sb16 += L1R[r]["nblocks"] // 16

            # ================= LAYER 2 scatter (2 rounds, transposed) ======
            ps2 = {}

            def l2_finalize(b):
                psb2 = ps2.pop(b)
                if sched["g2b_nz"]:
                    nc.tensor.matmul(
                        psb2[:, :], lhsT=g2b_s[:1, :],
                        rhs=sdeg_s[:1, b * P:(b + 1) * P], start=False,
                        stop=True, skip_group_check=True)
                g2T = qp.tile([G2, P], bf, tag="g2T")
                nc.scalar.activation(g2T[:, :], psb2[:, :], AF.Relu)
                psf = psW.tile([P, 512], f32, tag="w")
                nc.tensor.matmul(psf[:, :FOUT], lhsT=g2T[:, :],
                                 rhs=fcw_s[:, :], start=True, stop=True)
                nc.scalar.activation(
                    out_acc[:, b * FOUT:(b + 1) * FOUT], psf[:, :FOUT],
                    AF.Copy, scale=dlo_s[:, b:b + 1])
                if sched["fcb_nz"]:
                    nc.vector.tensor_add(
                        out_acc[:, b * FOUT:(b + 1) * FOUT],
                        out_acc[:, b * FOUT:(b + 1) * FOUT],
                        fcb_s[:, :])

            def l2_round(meta, sb_base32, table, is_b):
                for q in range(meta["nblocks"] // 32):
                    it = sb_base32 + q
                    gt2 = sp.tile([P, 32 * G2], f8, tag="gt2", bufs=2)
                    ix2 = sp.tile([P, 32], dt.int32, tag="ix2")
                    nc.sync.dma_start(out=ix2[:, :],
                                      in_=idx2_d[it * P:(it + 1) * P, :])
                    for jg in range(32):
                        if meta["skip"][q * 32 + jg]:
                            continue
                        nc.gpsimd.indirect_dma_start(
                            out=gt2[:, jg * G2:(jg + 1) * G2],
                            out_offset=None, in_=table[:, :],
                            in_offset=bass.IndirectOffsetOnAxis(
                                ap=ix2[:, jg:jg + 1], axis=0))
                    dr2 = sp.tile([P, 32], bf, tag="dr2")
                    nc.sync.dma_start(out=dr2[:, :],
                                      in_=drel2_d[it * P:(it + 1) * P, :])
                    S2 = sp.tile([P, 32 * P], f8, tag="S2", bufs=2)
                    nc.vector.tensor_tensor(
                        out=S2[:, :].rearrange("p (j c) -> p j c", c=P),
                        in0=iota_s[:, :].rearrange("p (j c) -> p j c", c=P),
                        in1=dr2[:, :].unsqueeze(2).to_broadcast([P, 32, P]),
                        op=OP.is_equal)
                    for j in range(32):
                        g = q * 32 + j
                        b = int(meta["b_of"][g])
                        first = bool(meta["first"][g])
                        last = bool(meta["last"][g])
                        if first:
                            psb2 = psS.tile([G2, P], f32, tag="agg",
                                            name="agg2")
                            ps2[b] = psb2
                            if is_b:
                                nc.tensor.matmul(
                                    psb2[:, :], lhsT=identb[:G2, :G2],
                                    rhs=agg2_s[:, b * P:(b + 1) * P],
                                    start=True, stop=False)
                        psb2 = ps2[b]
                        stop = last and (not sched["g2b_nz"] if is_b else True)
                        nc.tensor.matmul(
                            psb2[:, :], lhsT=gt2[:, j * G2:(j + 1) * G2],
                            rhs=S2[:, j * P:(j + 1) * P],
                            start=(first and not is_b), stop=stop)
                        if not last:
                            continue
                        if not is_b:
                            nc.vector.tensor_copy(
                                agg2_s[:, b * P:(b + 1) * P],
                                ps2.pop(b)[:, :])
                        else:
                            l2_finalize(b)

            l2_round(L2A, 0, h2gA, False)
            l2_round(L2B, L2A["nblocks"] // 32, h2gB, True)

            # batched log_softmax over all node blocks (logits are tiny:
            # exp without max-shift is safe)
            e_all = qp.tile([P, NB * FOUT], f32, tag="eall", bufs=1)
            nc.scalar.activation(e_all[:, :], out_acc[:, :], AF.Exp)
            sums = qp.tile([P, NB], f32, tag="sums", bufs=1)
            nc.vector.reduce_sum(
                sums[:, :],
                e_all[:, :].rearrange("p (b f) -> p b f", f=FOUT),
                axis=AX.X)
            lns = qp.tile([P, NB], f32, tag="lns", bufs=1)
            nc.scalar.activation(lns[:, :], sums[:, :], AF.Ln)
            res = qp.tile([P, NB * FOUT], f32, tag="eall", bufs=1, name="res")
            nc.vector.tensor_tensor(
                out=res[:, :].rearrange("p (b f) -> p b f", f=FOUT),
                in0=out_acc[:, :].rearrange("p (b f) -> p b f", f=FOUT),
                in1=lns[:, :].unsqueeze(2).to_broadcast([P, NB, FOUT]),
                op=OP.subtract)
            nc.scalar.dma_start(
                out=out_d[:, :].rearrange("(b p) f -> p b f", p=P),
                in_=res[:, :].rearrange("p (b f) -> p b f", f=FOUT))
            if debug_dump:
                for (srcten, dstten, tg) in [(h1t[0], dbg_t1, "d1"),
                                             (h1g[0], dbg_g1a, "d2")]:
                    tb = sp.tile([P, 4 * G1], f8, tag=tg, bufs=1)
                    nc.sync.dma_start(
                        out=tb[:, :].rearrange("p (a e) -> p a e", e=G1),
                        in_=srcten[:4 * P, :]
                            .rearrange("(a p) e -> p a e", p=P))
                    nc.sync.dma_start(
                        out=dstten[:, :].rearrange("(a p) e -> p a e", p=P),
                        in_=tb[:, :].rearrange("p (a e) -> p a e", e=G1))
                nc.sync.dma_start(out=dbg_z[:, :], in_=z_s[:, :])
                nc.sync.dma_start(out=dbg_sc[:, :NB], in_=dinv4_s[:, :])
                nc.sync.dma_start(out=dbg_sc[:, NB:2 * NB],
                                  in_=dinv16_s[:, :])
                nc.sync.dma_start(out=dbg_sc[:, 2 * NB:3 * NB],
                                  in_=dlo_s[:, :])
                nc.sync.dma_start(
                    out=dbg_sc[:, 3 * NB:3 * NB + BPC],
                    in_=rows1_s[:, :].bitcast(f32))
                gix = sp.tile([P, 64], dt.int32, tag="gix", bufs=1)
                nc.sync.dma_start(out=gix[:, :], in_=dbg_gidx[:, :])
                ggt = sp.tile([P, 64 * G2], f8, tag="ggt", bufs=1)
                for jg in range(64):
                    nc.gpsimd.indirect_dma_start(
                        out=ggt[:, jg * G2:(jg + 1) * G2], out_offset=None,
                        in_=h2gA[:, :],
                        in_offset=bass.IndirectOffsetOnAxis(
                            ap=gix[:, jg:jg + 1], axis=0))
                nc.sync.dma_start(out=dbg_gath[:, :], in_=ggt[:, :])
                nc.sync.dma_start(out=dbg_agg2[:, :], in_=agg2_s[:, :])
                nc.sync.dma_start(out=dbg_mt[:, :], in_=mT_s[:, :])
    return nc


_LAST_EXEC_NS = None
_LAST_RESULT = None


def run(inputs, cfg, trace=False, debug=False, debug_dump=False):
    global _LAST_EXEC_NS, _LAST_RESULT
    in_maps, sched = host_prep(inputs, cfg)
    nc = build(cfg, sched, debug=debug, debug_dump=debug_dump)
    nc.finalize()
    from concourse import bass_utils
    res = bass_utils.run_bass_kernel_spmd(
        nc, in_maps, core_ids=list(range(cfg.NC)), trace=trace)
    _LAST_EXEC_NS = res.exec_time_ns
    _LAST_RESULT = res
    outs = [np.asarray(res.results[c]["out"])[:cfg.NLOC_RAW]
            for c in range(cfg.NC)]
    return np.concatenate(outs, 0).astype(np.float32)


def kernel(**inputs):
    return run(inputs, _Cfg(**CFG_FULL))


# revision 27
# speedup vs baseline: 1.0166x; 1.0166x over previous
"""Distributed Trainium2 kernel for the GNN message-passing model.

Self-contained: host-side structural prep (sharding, edge sort, index
remap) + Bass/Tile SPMD kernel across 8 NeuronCores.

Math (see reference):
  logits = MLP(x1); m = 0.15 + 0.55*onehot(argmax(logits))
  r1 = (m@W1s)*x2 + m@bp1
  g1 = relu(Dh A Dh (r1@gcn1_w) + gcn1_b); g1 = (m@W12)*g1 + 2e-4*(r1@W13)
  r2 = (m@W2s)*g1 + m@bp2
  g2 = relu(Dh A Dh (r2@gcn2_w) + gcn2_b)
  out = log_softmax(g2@fc_w + fc_b)
where Dh = diag(deg^-1/2), deg = in-degree over dst.

Distribution: nodes sharded contiguously over 8 cores. Per GCN layer
the scaled features h' = Dh*h are exchanged in fp8 chunks; each core
then gathers h'[src] for edges whose dst it owns via indirect DMA and
scatter-reduces them with one-hot matmuls on the TensorEngine (PSUM
accumulation per dst block).

Environment-driven design notes (axon-tunneled trn2, fake_nrt):
- Only AllReduce(add) has correct collective semantics here
  (AllGather/ReduceScatter/AllToAll average). AllGather is emulated as
  zero-filled tables + per-rank indirect scatter into rank slots +
  AllReduce(add) over the disjoint contributions.
- Indirect DMA honors only ONE offset per partition per op; extra
  offset columns read base+j consecutive rows. All gathers are
  single-offset per 128-edge block; the producer scatter uses the
  base+consecutive form (table rows partition-major per rank slice) so
  one op writes a whole chunk.
- Indirect ops cost ~1.1us fixed each, so the per-edge-block gather
  stream (~1.7k ops) dominates the runtime.

fp8 scaling scheme (linear factors commute through relu/scatter):
  weights w1,w2,w3,g1w,W13,g2w scaled x16 host-side (avoids fp8
  subnormals); tables carry 64*dinv[src]*h; consumers unscale via the
  activation-scale slots (dinv/64).
Layer-2 scatter is transposed: psum[feat=32, dst=128] accumulates
lhsT=gathered values, rhs=one-hot masks (generated on-device from
per-edge drel via iota is_equal); relu + fc run directly on the
transposed tile and dinv[dst]/64 is folded into the final psum->out
copy (per-partition scale), eliminating per-block transposes.
"""

import numpy as np

P = 128
TAU_HI = 0.7
TAU_LO = 0.15  # (1-0.7)/2
SW = 16.0     # weight upscale (w1,w2,w3,g1w,W13,g2w)
TS = 64.0     # gathered-table scale (both layers)


class _Cfg:
    def __init__(self, N, E, F1=768, H=512, G1=256, G2=32, FOUT=40, C=7):
        self.NC = 8
        self.N = N
        self.E = E
        self.NLOC_RAW = N // self.NC
        self.NB = -(-self.NLOC_RAW // P)          # node blocks per core
        self.NLOC = self.NB * P
        assert self.NB % C == 0, (self.NB, C)
        self.C = C                                 # allgather chunks
        self.BPC = self.NB // C                    # blocks per chunk
        self.CH = self.BPC * P                     # chunk nodes
        self.TR = self.NC * self.NLOC              # gathered table rows
        self.CHR = self.NC * self.CH               # rows per chunk in table
        self.F1, self.H, self.G1, self.G2, self.FOUT = F1, H, G1, G2, FOUT
        self.KF1 = F1 // P                         # 6 k-tiles
        self.KH = H // P                           # 4
        self.KG1 = G1 // P                         # 2
        self.RB1 = [0, 6, C]                       # L1 round chunk bounds
        self.SPL2 = max(1, C - 3)                  # L2 round-A src chunks
        self.NFREE = min(448, self.CH)             # front free-dim unit
        assert self.CH % self.NFREE == 0
        self.FU = self.CH // self.NFREE            # free units per chunk


CFG_FULL = dict(N=50000, E=800000)


def _to_bf16(x):
    import ml_dtypes
    return np.asarray(x, np.float32).astype(ml_dtypes.bfloat16)


def _to_f8(x):
    import ml_dtypes
    return np.asarray(x, np.float32).astype(ml_dtypes.float8_e4m3)


def _row_of_node(v, cfg):
    """Gathered-table row for global node id v (vectorized).

    Layout within a (chunk k, rank c) slice is partition-major
    (row = p*BPC + nb) so the producer can scatter a whole chunk with a
    single base+consecutive-rows indirect DMA."""
    c = v // cfg.NLOC_RAW
    s = v - c * cfg.NLOC_RAW
    k = s // cfg.CH
    off = s - k * cfg.CH
    nb = off // P
    p = off - nb * P
    return k * cfg.CHR + c * cfg.CH + p * cfg.BPC + nb


def host_prep(inputs, cfg):
    """Returns (in_maps, sched). sched is baked into the built graph and
    must be identical for every core (SPMD)."""
    x1 = np.asarray(inputs["x1"], np.float32)
    x2 = np.asarray(inputs["x2"], np.float32)
    ei = np.asarray(inputs["edge_index"])
    src = ei[0].astype(np.int64)
    dst = ei[1].astype(np.int64)
    N, E, NC = cfg.N, cfg.E, cfg.NC
    assert x1.shape[0] == N and src.shape[0] == E

    deg = np.bincount(dst, minlength=N).astype(np.float64)
    dinv = np.where(deg > 0, deg ** -0.5, 0.0).astype(np.float32)
    sdeg = np.sqrt(deg).astype(np.float32)  # 1/dinv where deg>0 else 0

    # ---- per-core edge partition by dst owner, sorted by dst block ----
    owner = dst // cfg.NLOC_RAW
    dloc = dst - owner * cfg.NLOC_RAW
    dblk = dloc // P
    drel_all = (dloc - dblk * P).astype(np.float32)
    rows_all = _row_of_node(src, cfg).astype(np.int32)

    per_core = []
    cnt = np.zeros((NC, cfg.NB), np.int64)
    for c in range(NC):
        sel = np.where(owner == c)[0]
        order = np.argsort(dblk[sel], kind="stable")
        sel = sel[order]
        b_of = dblk[sel]
        bounds = np.searchsorted(b_of, np.arange(cfg.NB + 1))
        lists = []
        for b in range(cfg.NB):
            idxs = sel[bounds[b]:bounds[b + 1]]
            lists.append((rows_all[idxs], drel_all[idxs]))
            cnt[c, b] = len(idxs)
        per_core.append(lists)

    # Two uniform cross-core layouts, each split in 2 rounds by src chunk
    # (round boundary = which AllGather chunks the gathers depend on).
    def build_layout(chunk_bounds, pad_mult):
        bounds_k = [b * cfg.CHR for b in chunk_bounds]
        layout = dict(rounds=[])
        for r in range(len(chunk_bounds) - 1):
            lo, hi = bounds_k[r], bounds_k[r + 1]
            cntr = np.zeros((NC, cfg.NB), np.int64)
            per_rc = []
            for c in range(NC):
                pc = []
                for b in range(cfg.NB):
                    rows, rel = per_core[c][b]
                    m = (rows >= lo) & (rows < hi)
                    pc.append((rows[m] - lo, rel[m]))
                    cntr[c, b] = int(m.sum())
                per_rc.append(pc)
            Kb = np.maximum(1, -(-cntr.max(axis=0) // P)).astype(np.int64)
            nb_round = int(Kb.sum())
            pad = (-nb_round) % pad_mult
            nb_round += pad
            b_of = np.concatenate([np.repeat(np.arange(cfg.NB), Kb),
                                   np.full(pad, cfg.NB - 1)])
            first = np.zeros(nb_round, bool)
            last = np.zeros(nb_round, bool)
            skip = np.ones(nb_round, bool)
            off = 0
            for b in range(cfg.NB):
                first[off] = True
                e = off + int(Kb[b])
                if b == cfg.NB - 1:
                    e = nb_round
                last[e - 1] = True
                for jj in range(int(Kb[b])):
                    skip[off + jj] = not bool((cntr[:, b] > jj * P).any())
                off += int(Kb[b])
            layout["rounds"].append(dict(Kb=Kb, nblocks=nb_round, b_of=b_of,
                                         first=first, last=last, skip=skip,
                                         per_rc=per_rc))
        return layout

    lay1 = build_layout(cfg.RB1, 16)
    lay2 = build_layout([0, cfg.SPL2, cfg.C], 32)

    def pack_layout(layout, c, JW):
        idxs, Ss = [], []
        for rr in layout["rounds"]:
            sbs = rr["nblocks"] // 8
            idx = np.zeros((sbs * P, 8), np.int32)
            drl = np.full((sbs * P, 8), -1.0, np.float32)
            g = 0
            for b in range(cfg.NB):
                rows, rel = rr["per_rc"][c][b]
                n = len(rows)
                nblk = int(rr["Kb"][b])
                if b == cfg.NB - 1:
                    nblk = rr["nblocks"] - g
                for j in range(nblk):
                    s, jj = g // 8, g % 8
                    e0 = j * P
                    m = min(P, max(0, n - e0))
                    if m > 0:
                        idx[s * P:s * P + m, jj] = rows[e0:e0 + m]
                        drl[s * P:s * P + m, jj] = rel[e0:e0 + m]
                    g += 1
            # regroup [sbs*P, 8] -> [(nblocks//JW)*P, JW] so device loads
            # are plain 2D slices (per-iteration rows)
            g8 = JW // 8
            nq = sbs // g8
            idx = (idx.reshape(nq, g8, P, 8).transpose(0, 2, 1, 3)
                   .reshape(nq * P, JW))
            drl = (drl.reshape(nq, g8, P, 8).transpose(0, 2, 1, 3)
                   .reshape(nq * P, JW))
            idxs.append(idx.copy())
            Ss.append(_to_bf16(drl))
        return (np.concatenate(idxs, axis=0), np.concatenate(Ss, axis=0))

    def mk(lay):
        return [dict(nblocks=r["nblocks"], b_of=r["b_of"], first=r["first"],
                     last=r["last"], skip=r["skip"]) for r in lay["rounds"]]
    sched = dict(lay1=mk(lay1), lay2=mk(lay2))

    # ---- weights ----
    w1 = np.asarray(inputs["mlp_w1"], np.float32)
    w2 = np.asarray(inputs["mlp_w2"], np.float32)
    w3 = np.asarray(inputs["mlp_w3"], np.float32)
    b1 = np.asarray(inputs["mlp_b1"], np.float32)
    b2 = np.asarray(inputs["mlp_b2"], np.float32)
    b3 = np.asarray(inputs["mlp_b3"], np.float32)
    W1s = np.asarray(inputs["W1"], np.float32).sum(-1)
    W12 = np.asarray(inputs["W12"], np.float32)
    W13 = np.asarray(inputs["W13"], np.float32)  # 2e-4 folded on-device
    bp1 = np.asarray(inputs["bp1"], np.float32)
    W2s = np.asarray(inputs["W2"], np.float32).sum(-1)
    bp2 = np.asarray(inputs["bp2"], np.float32)
    g1w = np.asarray(inputs["gcn1_w"], np.float32)
    g1b = np.asarray(inputs["gcn1_b"], np.float32)
    g2w = np.asarray(inputs["gcn2_w"], np.float32)
    g2b = np.asarray(inputs["gcn2_b"], np.float32)
    fcw = np.asarray(inputs["fc_w"], np.float32)
    fcb = np.asarray(inputs["fc_b"], np.float32)

    sched["bp1_nz"] = bool(np.any(bp1 != 0))
    sched["bp2_nz"] = bool(np.any(bp2 != 0))
    sched["g1b_nz"] = bool(np.any(g1b != 0))
    sched["g2b_nz"] = bool(np.any(g2b != 0))
    sched["fcb_nz"] = bool(np.any(fcb != 0))
    sched["b3_nz"] = bool(np.any(b3 != 0))
    sched["w12_ones"] = bool(np.all(W12 == 1.0))

    def pack_lhsT(w, KT, MT):
        o = np.zeros((P, KT * MT * P), np.float32)
        for k in range(KT):
            for m in range(MT):
                o[:, (k * MT + m) * P:(k * MT + m + 1) * P] = \
                    w[k * P:(k + 1) * P, m * P:(m + 1) * P]
        return o

    def pack_rhs(w, KT, F):
        o = np.zeros((P, KT * F), np.float32)
        for k in range(KT):
            o[:, k * F:(k + 1) * F] = w[k * P:(k + 1) * P, :]
        return o

    def pack_k3(w, F):
        o = np.zeros((4, F), np.float32)
        o[:3] = w
        return _to_bf16(o)

    w1_p = _to_f8(pack_lhsT(w1 * SW, cfg.KF1, cfg.KH))
    w2_p = _to_f8(pack_lhsT(w2 * SW, cfg.KH, cfg.KH))
    w3_p = _to_f8(pack_rhs(np.pad(w3 * SW, ((0, 0), (0, 1))), cfg.KH, 4))
    b1_p = b1.reshape(cfg.KH, P).T.copy()
    b2_p = b2.reshape(cfg.KH, P).T.copy()
    b3_p = np.pad(b3 * SW, (0, 1)).reshape(1, 4).repeat(P, 0).copy()
    # fused gcn1 rhs: per k-slab [g1w*16 | W13*16] -> [P, KF1*2*G1]
    gw = np.zeros((P, cfg.KF1 * 2 * cfg.G1), np.float32)
    for k in range(cfg.KF1):
        gw[:, k * 2 * cfg.G1:k * 2 * cfg.G1 + cfg.G1] = \
            g1w[k * P:(k + 1) * P, :] * SW
        gw[:, k * 2 * cfg.G1 + cfg.G1:(k + 1) * 2 * cfg.G1] = \
            W13[k * P:(k + 1) * P, :] * SW
    gw_p = _to_f8(gw)
    g2w_p = _to_bf16(pack_rhs(g2w * SW, cfg.KG1, cfg.G2))
    fcw_p = _to_bf16(fcw)
    W1s_p = pack_k3(W1s, cfg.F1)
    bp1_p = pack_k3(bp1, cfg.F1)
    W12_p = pack_k3(W12, cfg.G1)
    W2s_p = pack_k3(W2s, cfg.G1)
    bp2_p = pack_k3(bp2, cfg.G1)
    g1b_p = _to_bf16(g1b.reshape(1, cfg.G1))
    g2b_p = _to_bf16(g2b.reshape(1, cfg.G2))
    fcb_p = np.repeat(fcb.reshape(1, cfg.FOUT), P, axis=0).astype(np.float32)

    in_maps = []
    for c in range(NC):
        lo = c * cfg.NLOC_RAW
        hi = lo + cfg.NLOC_RAW
        x1T = np.zeros((cfg.F1, cfg.NLOC), np.float32)
        x1T[:, :cfg.NLOC_RAW] = x1[lo:hi].T
        x2T = np.zeros((cfg.F1, cfg.NLOC), np.float32)
        x2T[:, :cfg.NLOC_RAW] = x2[lo:hi].T

        def dpack(v):
            t = np.zeros(cfg.NLOC, np.float32)
            t[:cfg.NLOC_RAW] = v[lo:hi]
            return t.reshape(cfg.NB, P).T.copy()

        dinv4_t = dpack(dinv * (TS / SW))
        dinv16_t = dpack(dinv * SW)
        dlo_t = dpack(dinv / TS)
        sdeg_r = np.zeros((1, cfg.NLOC), np.float32)
        sdeg_r[0, :cfg.NLOC_RAW] = sdeg[lo:hi] * TS

        idx1, drel1 = pack_layout(lay1, c, 16)
        idx2, drel2 = pack_layout(lay2, c, 32)
        rows1 = (c * cfg.CH
                 + np.arange(P, dtype=np.int32)[:, None] * cfg.BPC
                 ).astype(np.int32)
        iota = np.tile(np.arange(P, dtype=np.float32), 32).reshape(1, 32 * P)
        im = {
            "identb": _to_bf16(np.eye(P, dtype=np.float32)),
            "identf": _to_f8(np.eye(P, dtype=np.float32)),
            "iota": _to_bf16(np.repeat(iota, P, axis=0)),
            "x1T": _to_f8(x1T), "x2T": _to_bf16(x2T),
            "idx1": idx1, "drel1": drel1, "idx2": idx2, "drel2": drel2,
            "rows1": rows1,
            "dinv4": dinv4_t, "dinv16": dinv16_t, "dlo": dlo_t,
            "sdeg64": _to_bf16(sdeg_r),
            "w1": w1_p, "w2": w2_p, "w3": w3_p,
            "b1": b1_p, "b2": b2_p, "b3": b3_p,
            "gw": gw_p, "g2w": g2w_p, "fcw": fcw_p,
            "W1s": W1s_p, "bp1": bp1_p, "W12": W12_p, "W2s": W2s_p,
            "bp2": bp2_p, "g1b": g1b_p, "g2b": g2b_p, "fcb": fcb_p,
        }
        in_maps.append(im)
    return in_maps, sched


def build(cfg, sched, debug=False, debug_dump=False):
    import concourse.bacc as bacc
    import concourse.bass as bass
    import concourse.mybir as mybir
    import concourse.tile as tile

    dt = mybir.dt
    AF = mybir.ActivationFunctionType
    OP = mybir.AluOpType
    AX = mybir.AxisListType

    nc = bacc.Bacc("TRN2", target_bir_lowering=False, debug=debug)

    NB, C, BPC, CH, NLOC, TR, CHR = (cfg.NB, cfg.C, cfg.BPC, cfg.CH,
                                     cfg.NLOC, cfg.TR, cfg.CHR)
    F1, H, G1, G2, FOUT = cfg.F1, cfg.H, cfg.G1, cfg.G2, cfg.FOUT
    KF1, KH, KG1 = cfg.KF1, cfg.KH, cfg.KG1
    NF, FU = cfg.NFREE, cfg.FU
    L1R = sched["lay1"]
    L2A, L2B = sched["lay2"]
    SB1 = sum(r["nblocks"] for r in L1R) // 8
    SB2T = (L2A["nblocks"] + L2B["nblocks"]) // 8
    RB1, SPL2 = cfg.RB1, cfg.SPL2
    NR1 = len(RB1) - 1

    bf = dt.bfloat16
    f8 = dt.float8e4
    f32 = dt.float32

    dd = {}

    def din(name, shape, dtype):
        dd[name] = nc.declare_dram_parameter(name, list(shape), dtype,
                                             isOutput=False)
        return dd[name]

    x1T_d = din("x1T", [F1, NLOC], f8)
    x2T_d = din("x2T", [F1, NLOC], bf)
    idx1_d = din("idx1", [SB1 // 2 * P, 16], dt.int32)
    drel1_d = din("drel1", [SB1 // 2 * P, 16], bf)
    idx2_d = din("idx2", [SB2T // 4 * P, 32], dt.int32)
    drel2_d = din("drel2", [SB2T // 4 * P, 32], bf)
    iota_d = din("iota", [P, 32 * P], bf)
    rows1_d = din("rows1", [P, 1], dt.int32)
    dinv4_d = din("dinv4", [P, NB], f32)
    dinv16_d = din("dinv16", [P, NB], f32)
    dlo_d = din("dlo", [P, NB], f32)
    sdeg_d = din("sdeg64", [1, NLOC], bf)
    w1_d = din("w1", [P, KF1 * KH * P], f8)
    w2_d = din("w2", [P, KH * KH * P], f8)
    w3_d = din("w3", [P, KH * 4], f8)
    b1_d = din("b1", [P, KH], f32)
    b2_d = din("b2", [P, KH], f32)
    b3_d = din("b3", [P, 4], f32)
    gw_d = din("gw", [P, KF1 * 2 * G1], f8)
    g2w_d = din("g2w", [P, KG1 * G2], bf)
    fcw_d = din("fcw", [G2, FOUT], bf)
    W1s_d = din("W1s", [4, F1], bf)
    bp1_d = din("bp1", [4, F1], bf)
    W12_d = din("W12", [4, G1], bf)
    W2s_d = din("W2s", [4, G1], bf)
    bp2_d = din("bp2", [4, G1], bf)
    g1b_d = din("g1b", [1, G1], bf)
    g2b_d = din("g2b", [1, G2], bf)
    fcb_d = din("fcb", [P, FOUT], f32)
    identb_d = din("identb", [P, P], bf)
    identf_d = din("identf", [P, P], f8)
    out_d = nc.declare_dram_parameter("out", [NLOC, FOUT], f32, isOutput=True)
    if debug_dump:
        dbg_gath = nc.declare_dram_parameter("dbg_gath", [P, 64 * G2], f8,
                                             isOutput=True)
        dbg_t1 = nc.declare_dram_parameter("dbg_t1", [4 * P, G1], f8,
                                           isOutput=True)
        dbg_z = nc.declare_dram_parameter("dbg_z", [P, NB * G1], bf,
                                          isOutput=True)
        dbg_sc = nc.declare_dram_parameter("dbg_sc", [P, 4 * NB], f32,
                                           isOutput=True)
        dbg_stg = nc.declare_dram_parameter("dbg_stg", [P, BPC * G1], f8,
                                            isOutput=True)
        dbg_g1a = nc.declare_dram_parameter("dbg_g1a", [4 * P, G1], f8,
                                            isOutput=True)
        dbg_gidx = nc.declare_dram_parameter("dbg_gidx", [P, 64], dt.int32,
                                             isOutput=False)
        dbg_agg2 = nc.declare_dram_parameter("dbg_agg2", [G2, NB * P], bf,
                                             isOutput=True)
        dbg_mt = nc.declare_dram_parameter("dbg_mt", [4, NLOC], bf,
                                           isOutput=True)

    with tile.TileContext(nc) as tc:
        with (
            tc.tile_pool(name="const", bufs=1) as cp,
            tc.tile_pool(name="front", bufs=2) as fp,
            tc.tile_pool(name="scat", bufs=3) as sp,
            tc.tile_pool(name="fin", bufs=2) as qp,
            tc.tile_pool(name="psG", bufs=2, space="PSUM") as psG,
            tc.tile_pool(name="psS", bufs=2, space="PSUM") as psS,
            tc.tile_pool(name="psW", bufs=2, space="PSUM") as psW,
            tc.tile_pool(name="psT", bufs=2, space="PSUM") as psT,
            tc.tile_pool(name="dram", bufs=1, space="DRAM") as dp,
        ):
            def load(dr, shape, dtype, name):
                t = cp.tile(shape, dtype, tag=name)
                nc.sync.dma_start(out=t[:, :], in_=dr[:, :])
                return t

            w1_s = load(w1_d, [P, KF1 * KH * P], f8, "w1")
            w2_s = load(w2_d, [P, KH * KH * P], f8, "w2")
            w3_s = load(w3_d, [P, KH * 4], f8, "w3")
            b1_s = load(b1_d, [P, KH], f32, "b1")
            b2_s = load(b2_d, [P, KH], f32, "b2")
            b3_s = load(b3_d, [P, 4], f32, "b3")
            gw_s = load(gw_d, [P, KF1 * 2 * G1], f8, "gw")
            g2w_s = load(g2w_d, [P, KG1 * G2], bf, "g2w")
            fcw_s = load(fcw_d, [G2, FOUT], bf, "fcw")
            W1s_s = load(W1s_d, [4, F1], bf, "W1s")
            bp1_s = load(bp1_d, [4, F1], bf, "bp1")
            W12_s = load(W12_d, [4, G1], bf, "W12")
            W2s_s = load(W2s_d, [4, G1], bf, "W2s")
            bp2_s = load(bp2_d, [4, G1], bf, "bp2")
            g1b_s = load(g1b_d, [1, G1], bf, "g1b")
            g2b_s = load(g2b_d, [1, G2], bf, "g2b")
            fcb_s = load(fcb_d, [P, FOUT], f32, "fcb")
            dinv4_s = load(dinv4_d, [P, NB], f32, "dinv4")
            dinv16_s = load(dinv16_d, [P, NB], f32, "dinv16")
            dlo_s = load(dlo_d, [P, NB], f32, "dlo")
            sdeg_s = load(sdeg_d, [1, NLOC], bf, "sdeg")

            identb = load(identb_d, [P, P], bf, "identb")
            identf = load(identf_d, [P, P], f8, "identf")
            iota_s = load(iota_d, [P, 32 * P], bf, "iota")
            rows1_s = load(rows1_d, [P, 1], dt.int32, "rows1")
            ztile = cp.tile([P, 14 * G1], f8, tag="ztile")
            nc.vector.memset(ztile[:, :], 0.0)

            mT_s = cp.tile([4, NLOC], bf, tag="mT")
            out_acc = cp.tile([P, NB * FOUT], f32, tag="oacc")
            z_s = cp.tile([P, NB * G1], bf, tag="z")
            aggA_s = cp.tile([P, NB * G1], bf, tag="aggA")
            agg2_s = cp.tile([G2, NB * P], bf, tag="agg2")

            h1t = [dp.tile([CHR, G1], f8, tag=f"h1t{k}", name=f"h1t{k}")
                   for k in range(C)]
            h2t = [dp.tile([CHR, G2], f8, tag=f"h2t{k}", name=f"h2t{k}")
                   for k in range(C)]
            h2stg = cp.tile([P, NB * G2], f8, tag="h2stg")
            # zero-fill AR input tables upfront (remote slots must be 0;
            # AllReduce(add) over disjoint slots emulates AllGather, which
            # has broken semantics in this runtime)
            ZB = CHR // P // 4
            for k in range(C):
                for z4 in range(4):
                    nc.sync.dma_start(
                        out=h1t[k][z4 * ZB * P:(z4 + 1) * ZB * P, :]
                            .rearrange("(a p) e -> p a e", p=P),
                        in_=ztile[:, :ZB * G1]
                            .rearrange("p (a e) -> p a e", e=G1))
            for k in range(C):
                for z4 in range(4):
                    nc.sync.dma_start(
                        out=h2t[k][z4 * ZB * P:(z4 + 1) * ZB * P, :]
                            .rearrange("(a p) e -> p a e", p=P),
                        in_=ztile[:, :ZB * G2]
                            .rearrange("p (a e) -> p a e", e=G2))
            h1g = [dp.tile([(RB1[r + 1] - RB1[r]) * CHR, G1], f8,
                           tag=f"h1g{r}", name=f"h1g{r}")
                   for r in range(NR1)]
            h2gA = dp.tile([SPL2 * CHR, G2], f8, tag="h2gA")
            h2gB = dp.tile([(C - SPL2) * CHR, G2], f8, tag="h2gB")

            # ================= FRONT (per chunk) =================
            for k in range(C):
                n0 = k * CH
                x1c = fp.tile([P, KF1 * CH], f8, tag="x1c")
                nc.sync.dma_start(
                    out=x1c[:, :].rearrange("p (a n) -> p a n", n=CH),
                    in_=x1T_d[:, n0:n0 + CH].rearrange("(a p) n -> p a n", p=P))
                x2c = fp.tile([P, KF1 * CH], bf, tag="x2c", bufs=1)
                nc.sync.dma_start(
                    out=x2c[:, :].rearrange("p (a n) -> p a n", n=CH),
                    in_=x2T_d[:, n0:n0 + CH].rearrange("(a p) n -> p a n", p=P))

                h1T = fp.tile([P, KH * CH], f8, tag="h1T", bufs=1)
                for u in range(FU):
                    for m in range(KH):
                        ps = psG.tile([P, NF], f32, tag="g")
                        for kk in range(KF1):
                            nc.tensor.matmul(
                                ps[:, :],
                                lhsT=w1_s[:, (kk * KH + m) * P:(kk * KH + m + 1) * P],
                                rhs=x1c[:, kk * CH + u * NF:kk * CH + u * NF + NF],
                                start=(kk == 0), stop=(kk == KF1 - 1))
                        nc.scalar.activation(
                            h1T[:, m * CH + u * NF:m * CH + u * NF + NF],
                            ps[:, :], AF.Relu, bias=b1_s[:, m:m + 1],
                            scale=1.0 / SW)
                h2T = fp.tile([P, KH * CH], f8, tag="h2T", bufs=1)
                for u in range(FU):
                    for m in range(KH):
                        ps = psG.tile([P, NF], f32, tag="g")
                        for kk in range(KH):
                            nc.tensor.matmul(
                                ps[:, :],
                                lhsT=w2_s[:, (kk * KH + m) * P:(kk * KH + m + 1) * P],
                                rhs=h1T[:, kk * CH + u * NF:kk * CH + u * NF + NF],
                                start=(kk == 0), stop=(kk == KH - 1))
                        nc.scalar.activation(
                            h2T[:, m * CH + u * NF:m * CH + u * NF + NF],
                            ps[:, :], AF.Relu, bias=b2_s[:, m:m + 1],
                            scale=1.0 / SW)

                mmc = fp.tile([P, BPC * 3], bf, tag="mmc")
                for nb in range(BPC):
                    psl = psW.tile([P, 512], f32, tag="w")
                    for kk in range(KH):
                        nc.tensor.matmul(
                            psl[:, :4],
                            lhsT=h2T[:, kk * CH + nb * P:kk * CH + (nb + 1) * P],
                            rhs=w3_s[:, kk * 4:(kk + 1) * 4],
                            start=(kk == 0), stop=(kk == KH - 1))
                    lg = fp.tile([P, 3], f32, tag="lg")
                    if sched["b3_nz"]:
                        nc.vector.tensor_add(lg[:, :], psl[:, :3], b3_s[:, :3])
                    else:
                        nc.vector.tensor_copy(lg[:, :], psl[:, :3])
                    rmax = fp.tile([P, 1], f32, tag="rmax")
                    nc.vector.reduce_max(rmax[:, :], lg[:, :], axis=AX.X)
                    mm = fp.tile([P, 3], bf, tag="mm")
                    nc.vector.tensor_scalar(
                        mm[:, :], lg[:, :], rmax[:, :1], None, OP.is_equal)
                    nc.scalar.activation(mmc[:, nb * 3:(nb + 1) * 3],
                                         mm[:, :], AF.Copy,
                                         bias=TAU_LO, scale=TAU_HI - TAU_LO)
                for nb in range(BPC):
                    b_glob = k * BPC + nb
                    pst = psT.tile([P, P], bf, tag="t")
                    nc.tensor.transpose(pst[:3, :],
                                        mmc[:, nb * 3:(nb + 1) * 3],
                                        identb[:, :])
                    nc.vector.tensor_copy(
                        mT_s[:3, b_glob * P:(b_glob + 1) * P], pst[:3, :])

                r1T = fp.tile([P, KF1 * CH], f8, tag="r1T")
                for u in range(FU):
                    for f in range(KF1):
                        psr = psG.tile([P, NF], f32, tag="g")
                        nc.tensor.matmul(
                            psr[:, :], lhsT=W1s_s[:3, f * P:(f + 1) * P],
                            rhs=mT_s[:3, n0 + u * NF:n0 + u * NF + NF],
                            start=True, stop=True)
                        if sched["bp1_nz"]:
                            psr2 = psW.tile([P, 512], f32, tag="w")
                            nc.tensor.matmul(
                                psr2[:, :NF], lhsT=bp1_s[:3, f * P:(f + 1) * P],
                                rhs=mT_s[:3, n0 + u * NF:n0 + u * NF + NF],
                                start=True, stop=True)
                            tmp = fp.tile([P, NF], f32, tag="r1tmp")
                            nc.vector.tensor_mul(
                                tmp[:, :], psr[:, :],
                                x2c[:, f * CH + u * NF:f * CH + u * NF + NF])
                            nc.vector.tensor_add(
                                r1T[:, f * CH + u * NF:f * CH + u * NF + NF],
                                tmp[:, :], psr2[:, :NF])
                        else:
                            nc.vector.tensor_mul(
                                r1T[:, f * CH + u * NF:f * CH + u * NF + NF],
                                psr[:, :],
                                x2c[:, f * CH + u * NF:f * CH + u * NF + NF])

                h1stg = fp.tile([P, BPC * G1], f8, tag="h1stg")
                for nb in range(BPC):
                    b_glob = k * BPC + nb
                    psh = psW.tile([P, 512], f32, tag="w")
                    for f in range(KF1):
                        nc.tensor.matmul(
                            psh[:, :],
                            lhsT=r1T[:, f * CH + nb * P:f * CH + (nb + 1) * P],
                            rhs=gw_s[:, f * 2 * G1:(f + 1) * 2 * G1],
                            start=(f == 0), stop=(f == KF1 - 1))
                    nc.scalar.activation(h1stg[:, nb * G1:(nb + 1) * G1],
                                         psh[:, :G1], AF.Copy,
                                         scale=dinv4_s[:, b_glob:b_glob + 1])
                    nc.scalar.activation(
                        z_s[:, b_glob * G1:(b_glob + 1) * G1],
                        psh[:, G1:2 * G1], AF.Copy, scale=2e-4 / SW)

                if debug_dump and k == C - 1:
                    nc.sync.dma_start(out=dbg_stg[:, :], in_=h1stg[:, :])
                nc.gpsimd.indirect_dma_start(
                    out=h1t[k][:, :],
                    out_offset=bass.IndirectOffsetOnAxis(
                        ap=rows1_s[:, :], axis=0),
                    in_=h1stg[:, :], in_offset=None)
                r1r = next(r for r in range(NR1)
                           if RB1[r] <= k < RB1[r + 1])
                kk0 = k - RB1[r1r]
                agt = h1g[r1r][kk0 * CHR:(kk0 + 1) * CHR, :]
                nc.gpsimd.collective_compute(
                    "AllReduce", OP.add,
                    replica_groups=[list(range(cfg.NC))],
                    ins=[h1t[k][:, :].opt()],
                    outs=[agt.opt()])

            # ================= LAYER 1 scatter (2 rounds) =================
            ps_by_b = {}

            def l1_finalize(b):
                psb = ps_by_b.pop(b)
                if sched["g1b_nz"]:
                    nc.tensor.matmul(
                        psb[:, :], lhsT=sdeg_s[:1, b * P:(b + 1) * P],
                        rhs=g1b_s[:1, :], start=False, stop=True,
                        skip_group_check=True)
                g1r = qp.tile([P, G1], bf, tag="g1r", bufs=3)
                nc.scalar.activation(g1r[:, :], psb[:, :], AF.Relu,
                                     scale=dlo_s[:, b:b + 1])
                psmw = psW.tile([P, 512], f32, tag="w")
                if not sched["w12_ones"]:
                    nc.tensor.matmul(psmw[:, :G1],
                                     lhsT=mT_s[:3, b * P:(b + 1) * P],
                                     rhs=W12_s[:3, :], start=True, stop=True)
                nc.tensor.matmul(psmw[:, G1:2 * G1],
                                 lhsT=mT_s[:3, b * P:(b + 1) * P],
                                 rhs=W2s_s[:3, :], start=True, stop=True)
                g1v = qp.tile([P, G1], bf, tag="g1v", bufs=3)
                if sched["w12_ones"]:
                    nc.vector.tensor_add(g1v[:, :], g1r[:, :],
                                         z_s[:, b * G1:(b + 1) * G1])
                else:
                    g1t = qp.tile([P, G1], bf, tag="g1t", bufs=3)
                    nc.vector.tensor_mul(g1t[:, :], g1r[:, :], psmw[:, :G1])
                    nc.vector.tensor_add(g1v[:, :], g1t[:, :],
                                         z_s[:, b * G1:(b + 1) * G1])
                r2 = qp.tile([P, G1], bf, tag="r2", bufs=3)
                if sched["bp2_nz"]:
                    psm3 = psW.tile([P, 512], f32, tag="w")
                    nc.tensor.matmul(psm3[:, :G1],
                                     lhsT=mT_s[:3, b * P:(b + 1) * P],
                                     rhs=bp2_s[:3, :], start=True, stop=True)
                    r2u = qp.tile([P, G1], f32, tag="r2u")
                    nc.vector.tensor_mul(r2u[:, :], g1v[:, :],
                                         psmw[:, G1:2 * G1])
                    r2v = qp.tile([P, G1], f32, tag="r2v")
                    nc.vector.tensor_add(r2v[:, :], r2u[:, :], psm3[:, :G1])
                    nc.vector.tensor_scalar(r2[:, :], r2v[:, :],
                                            dinv16_s[:, b:b + 1], None,
                                            OP.mult)
                else:
                    nc.vector.scalar_tensor_tensor(
                        out=r2[:, :], in0=g1v[:, :],
                        scalar=dinv16_s[:, b:b + 1],
                        in1=psmw[:, G1:2 * G1], op0=OP.mult, op1=OP.mult)
                r2T = qp.tile([P, KG1 * P], bf, tag="r2T", bufs=3)
                for f in range(KG1):
                    pst = psT.tile([P, P], bf, tag="t")
                    nc.tensor.transpose(pst[:, :], r2[:, f * P:(f + 1) * P],
                                        identb[:, :])
                    nc.vector.tensor_copy(r2T[:, f * P:(f + 1) * P],
                                          pst[:, :])
                psh2 = psW.tile([P, 512], f32, tag="w")
                for f in range(KG1):
                    nc.tensor.matmul(
                        psh2[:, :G2], lhsT=r2T[:, f * P:(f + 1) * P],
                        rhs=g2w_s[:, f * G2:(f + 1) * G2],
                        start=(f == 0), stop=(f == KG1 - 1))
                nc.scalar.activation(h2stg[:, b * G2:(b + 1) * G2],
                                     psh2[:, :G2], AF.Copy,
                                     scale=1.0 / 4.0)
                k, nb = b // BPC, b % BPC
                if nb == BPC - 1:
                    nc.gpsimd.indirect_dma_start(
                        out=h2t[k][:, :],
                        out_offset=bass.IndirectOffsetOnAxis(
                            ap=rows1_s[:, :], axis=0),
                        in_=h2stg[:, k * BPC * G2:(k + 1) * BPC * G2],
                        in_offset=None)
                    agt2 = (h2gA[k * CHR:(k + 1) * CHR, :] if k < SPL2 else
                            h2gB[(k - SPL2) * CHR:(k - SPL2 + 1) * CHR, :])
                    nc.gpsimd.collective_compute(
                        "AllReduce", OP.add,
                        replica_groups=[list(range(cfg.NC))],
                        ins=[h2t[k][:, :].opt()], outs=[agt2.opt()])

            def l1_round(meta, sb_base16, table, is_first, is_last):
                for s_loc in range(meta["nblocks"] // 16):
                    it = sb_base16 + s_loc
                    gt = sp.tile([P, 16 * G1], f8, tag="gt1", bufs=3)
                    ix = sp.tile([P, 16], dt.int32, tag="ix1")
                    nc.sync.dma_start(out=ix[:, :],
                                      in_=idx1_d[it * P:(it + 1) * P, :])
                    for jg in range(16):
                        if meta["skip"][s_loc * 16 + jg]:
                            continue
                        nc.gpsimd.indirect_dma_start(
                            out=gt[:, jg * G1:(jg + 1) * G1],
                            out_offset=None, in_=table[:, :],
                            in_offset=bass.IndirectOffsetOnAxis(
                                ap=ix[:, jg:jg + 1], axis=0))
                    dr = sp.tile([P, 16], bf, tag="dr1")
                    nc.sync.dma_start(out=dr[:, :],
                                      in_=drel1_d[it * P:(it + 1) * P, :])
                    Ssb = sp.tile([P, 16 * P], f8, tag="S1", bufs=2)
                    nc.vector.tensor_tensor(
                        out=Ssb[:, :].rearrange("p (j c) -> p j c", c=P),
                        in0=iota_s[:, :16 * P].rearrange("p (j c) -> p j c",
                                                         c=P),
                        in1=dr[:, :].unsqueeze(2).to_broadcast([P, 16, P]),
                        op=OP.is_equal)
                    for j in range(16):
                        g = s_loc * 16 + j
                        b = int(meta["b_of"][g])
                        first = bool(meta["first"][g])
                        last = bool(meta["last"][g])
                        if first:
                            psb = psS.tile([P, G1], f32, tag="agg",
                                           name="agg1")
                            ps_by_b[b] = psb
                            if not is_first:
                                nc.tensor.matmul(
                                    psb[:, :], lhsT=identb[:, :],
                                    rhs=aggA_s[:, b * G1:(b + 1) * G1],
                                    start=True, stop=False)
                        psb = ps_by_b[b]
                        stop = last and (not sched["g1b_nz"]
                                         if is_last else True)
                        nc.tensor.matmul(
                            psb[:, :], lhsT=Ssb[:, j * P:(j + 1) * P],
                            rhs=gt[:, j * G1:(j + 1) * G1],
                            start=(first and is_first), stop=stop)
                        if not last:
                            continue
                        if not is_last:
                            nc.vector.tensor_copy(
                                aggA_s[:, b * G1:(b + 1) * G1],
                                ps_by_b.pop(b)[:, :])
                        else:
                            l1_finalize(b)

            sb16 = 0
            for r in range(NR1):
                l1_round(L1R[r], sb16, h1g[r], r == 0, r == NR1 - 1)
                sb16 += L1R[r]["nblocks"] // 16

            # ================= LAYER 2 scatter (2 rounds, transposed) ======
            ps2 = {}

            def l2_finalize(b):
                psb2 = ps2.pop(b)
                if sched["g2b_nz"]:
                    nc.tensor.matmul(
                        psb2[:, :], lhsT=g2b_s[:1, :],
                        rhs=sdeg_s[:1, b * P:(b + 1) * P], start=False,
                        stop=True, skip_group_check=True)
                g2T = qp.tile([G2, P], bf, tag="g2T")
                nc.scalar.activation(g2T[:, :], psb2[:, :], AF.Relu)
                psf = psW.tile([P, 512], f32, tag="w")
                nc.tensor.matmul(psf[:, :FOUT], lhsT=g2T[:, :],
                                 rhs=fcw_s[:, :], start=True, stop=True)
                nc.scalar.activation(
                    out_acc[:, b * FOUT:(b + 1) * FOUT], psf[:, :FOUT],
                    AF.Copy, scale=dlo_s[:, b:b + 1])
                if sched["fcb_nz"]:
                    nc.vector.tensor_add(
                        out_acc[:, b * FOUT:(b + 1) * FOUT],
                        out_acc[:, b * FOUT:(b + 1) * FOUT],
                        fcb_s[:, :])

            def l2_round(meta, sb_base32, table, is_b):
                for q in range(meta["nblocks"] // 32):
                    it = sb_base32 + q
                    gt2 = sp.tile([P, 32 * G2], f8, tag="gt2", bufs=3)
                    ix2 = sp.tile([P, 32], dt.int32, tag="ix2")
                    nc.sync.dma_start(out=ix2[:, :],
                                      in_=idx2_d[it * P:(it + 1) * P, :])
                    for jg in range(32):
                        if meta["skip"][q * 32 + jg]:
                            continue
                        nc.gpsimd.indirect_dma_start(
                            out=gt2[:, jg * G2:(jg + 1) * G2],
                            out_offset=None, in_=table[:, :],
                            in_offset=bass.IndirectOffsetOnAxis(
                                ap=ix2[:, jg:jg + 1], axis=0))
                    dr2 = sp.tile([P, 32], bf, tag="dr2")
                    nc.sync.dma_start(out=dr2[:, :],
                                      in_=drel2_d[it * P:(it + 1) * P, :])
                    S2 = sp.tile([P, 32 * P], f8, tag="S2", bufs=2)
                    nc.vector.tensor_tensor(
                        out=S2[:, :].rearrange("p (j c) -> p j c", c=P),
                        in0=iota_s[:, :].rearrange("p (j c) -> p j c", c=P),
                        in1=dr2[:, :].unsqueeze(2).to_broadcast([P, 32, P]),
                        op=OP.is_equal)
                    for j in range(32):
                        g = q * 32 + j
                        b = int(meta["b_of"][g])
                        first = bool(meta["first"][g])
                        last = bool(meta["last"][g])
                        if first:
                            psb2 = psS.tile([G2, P], f32, tag="agg",
                                            name="agg2")
                            ps2[b] = psb2
                            if is_b:
                                nc.tensor.matmul(
                                    psb2[:, :], lhsT=identb[:G2, :G2],
                                    rhs=agg2_s[:, b * P:(b + 1) * P],
                                    start=True, stop=False)
                        psb2 = ps2[b]
                        stop = last and (not sched["g2b_nz"] if is_b else True)
                        nc.tensor.matmul(
                            psb2[:, :], lhsT=gt2[:, j * G2:(j + 1) * G2],
                            rhs=S2[:, j * P:(j + 1) * P],
                            start=(first and not is_b), stop=stop)
                        if not last:
                            continue
                        if not is_b:
                            nc.vector.tensor_copy(
                                agg2_s[:, b * P:(b + 1) * P],
                                ps2.pop(b)[:, :])
                        else:
                            l2_finalize(b)

            l2_round(L2A, 0, h2gA, False)
            l2_round(L2B, L2A["nblocks"] // 32, h2gB, True)

            # batched log_softmax over all node blocks (logits are tiny:
            # exp without max-shift is safe)
            e_all = qp.tile([P, NB * FOUT], f32, tag="eall", bufs=1)
            nc.scalar.activation(e_all[:, :], out_acc[:, :], AF.Exp)
            sums = qp.tile([P, NB], f32, tag="sums", bufs=1)
            nc.vector.reduce_sum(
                sums[:, :],
                e_all[:, :].rearrange("p (b f) -> p b f", f=FOUT),
                axis=AX.X)
            lns = qp.tile([P, NB], f32, tag="lns", bufs=1)
            nc.scalar.activation(lns[:, :], sums[:, :], AF.Ln)
            res = qp.tile([P, NB * FOUT], f32, tag="eall", bufs=1, name="res")
            nc.vector.tensor_tensor(
                out=res[:, :].rearrange("p (b f) -> p b f", f=FOUT),
                in0=out_acc[:, :].rearrange("p (b f) -> p b f", f=FOUT),
                in1=lns[:, :].unsqueeze(2).to_broadcast([P, NB, FOUT]),
                op=OP.subtract)
            nc.scalar.dma_start(
                out=out_d[:, :].rearrange("(b p) f -> p b f", p=P),
                in_=res[:, :].rearrange("p (b f) -> p b f", f=FOUT))
            if debug_dump:
                for (srcten, dstten, tg) in [(h1t[0], dbg_t1, "d1"),
                                             (h1g[0], dbg_g1a, "d2")]:
                    tb = sp.tile([P, 4 * G1], f8, tag=tg, bufs=1)
                    nc.sync.dma_start(
                        out=tb[:, :].rearrange("p (a e) -> p a e", e=G1),
                        in_=srcten[:4 * P, :]
                            .rearrange("(a p) e -> p a e", p=P))
                    nc.sync.dma_start(
                        out=dstten[:, :].rearrange("(a p) e -> p a e", p=P),
                        in_=tb[:, :].rearrange("p (a e) -> p a e", e=G1))
                nc.sync.dma_start(out=dbg_z[:, :], in_=z_s[:, :])
                nc.sync.dma_start(out=dbg_sc[:, :NB], in_=dinv4_s[:, :])
                nc.sync.dma_start(out=dbg_sc[:, NB:2 * NB],
                                  in_=dinv16_s[:, :])
                nc.sync.dma_start(out=dbg_sc[:, 2 * NB:3 * NB],
                                  in_=dlo_s[:, :])
                nc.sync.dma_start(
                    out=dbg_sc[:, 3 * NB:3 * NB + BPC],
                    in_=rows1_s[:, :].bitcast(f32))
                gix = sp.tile([P, 64], dt.int32, tag="gix", bufs=1)
                nc.sync.dma_start(out=gix[:, :], in_=dbg_gidx[:, :])
                ggt = sp.tile([P, 64 * G2], f8, tag="ggt", bufs=1)
                for jg in range(64):
                    nc.gpsimd.indirect_dma_start(
                        out=ggt[:, jg * G2:(jg + 1) * G2], out_offset=None,
                        in_=h2gA[:, :],
                        in_offset=bass.IndirectOffsetOnAxis(
                            ap=gix[:, jg:jg + 1], axis=0))
                nc.sync.dma_start(out=dbg_gath[:, :], in_=ggt[:, :])
                nc.sync.dma_start(out=dbg_agg2[:, :], in_=agg2_s[:, :])
                nc.sync.dma_start(out=dbg_mt[:, :], in_=mT_s[:, :])
    return nc


_LAST_EXEC_NS = None
_LAST_RESULT = None


def run(inputs, cfg, trace=False, debug=False, debug_dump=False):
    global _LAST_EXEC_NS, _LAST_RESULT
    in_maps, sched = host_prep(inputs, cfg)
    nc = build(cfg, sched, debug=debug, debug_dump=debug_dump)
    nc.finalize()
    from concourse import bass_utils
    res = bass_utils.run_bass_kernel_spmd(
        nc, in_maps, core_ids=list(range(cfg.NC)), trace=trace)
    _LAST_EXEC_NS = res.exec_time_ns
    _LAST_RESULT = res
    outs = [np.asarray(res.results[c]["out"])[:cfg.NLOC_RAW]
            for c in range(cfg.NC)]
    return np.concatenate(outs, 0).astype(np.float32)


def kernel(**inputs):
    return run(inputs, _Cfg(**CFG_FULL))


# revision 29
# speedup vs baseline: 1.0182x; 1.0016x over previous
"""Distributed Trainium2 kernel for the GNN message-passing model.

Self-contained: host-side structural prep (sharding, edge sort, index
remap) + Bass/Tile SPMD kernel across 8 NeuronCores.

Math (see reference):
  logits = MLP(x1); m = 0.15 + 0.55*onehot(argmax(logits))
  r1 = (m@W1s)*x2 + m@bp1
  g1 = relu(Dh A Dh (r1@gcn1_w) + gcn1_b); g1 = (m@W12)*g1 + 2e-4*(r1@W13)
  r2 = (m@W2s)*g1 + m@bp2
  g2 = relu(Dh A Dh (r2@gcn2_w) + gcn2_b)
  out = log_softmax(g2@fc_w + fc_b)
where Dh = diag(deg^-1/2), deg = in-degree over dst.

Distribution: nodes sharded contiguously over 8 cores. Per GCN layer
the scaled features h' = Dh*h are exchanged in fp8 chunks; each core
then gathers h'[src] for edges whose dst it owns via indirect DMA and
scatter-reduces them with one-hot matmuls on the TensorEngine (PSUM
accumulation per dst block).

Environment-driven design notes (axon-tunneled trn2, fake_nrt):
- Only AllReduce(add) has correct collective semantics here
  (AllGather/ReduceScatter/AllToAll average). AllGather is emulated as
  zero-filled tables + per-rank indirect scatter into rank slots +
  AllReduce(add) over the disjoint contributions.
- Indirect DMA honors only ONE offset per partition per op; extra
  offset columns read base+j consecutive rows. All gathers are
  single-offset per 128-edge block; the producer scatter uses the
  base+consecutive form (table rows partition-major per rank slice) so
  one op writes a whole chunk.
- Indirect ops cost ~1.1us fixed each, so the per-edge-block gather
  stream (~1.7k ops) dominates the runtime.

fp8 scaling scheme (linear factors commute through relu/scatter):
  weights w1,w2,w3,g1w,W13,g2w scaled x16 host-side (avoids fp8
  subnormals); tables carry 64*dinv[src]*h; consumers unscale via the
  activation-scale slots (dinv/64).
Layer-2 scatter is transposed: psum[feat=32, dst=128] accumulates
lhsT=gathered values, rhs=one-hot masks (generated on-device from
per-edge drel via iota is_equal); relu + fc run directly on the
transposed tile and dinv[dst]/64 is folded into the final psum->out
copy (per-partition scale), eliminating per-block transposes.
"""

import numpy as np

P = 128
TAU_HI = 0.7
TAU_LO = 0.15  # (1-0.7)/2
SW = 16.0     # weight upscale (w1,w2,w3,g1w,W13,g2w)
TS = 64.0     # gathered-table scale (both layers)


class _Cfg:
    def __init__(self, N, E, F1=768, H=512, G1=256, G2=32, FOUT=40, C=7):
        self.NC = 8
        self.N = N
        self.E = E
        self.NLOC_RAW = N // self.NC
        self.NB = -(-self.NLOC_RAW // P)          # node blocks per core
        self.NLOC = self.NB * P
        assert self.NB % C == 0, (self.NB, C)
        self.C = C                                 # allgather chunks
        self.BPC = self.NB // C                    # blocks per chunk
        self.CH = self.BPC * P                     # chunk nodes
        self.TR = self.NC * self.NLOC              # gathered table rows
        self.CHR = self.NC * self.CH               # rows per chunk in table
        self.F1, self.H, self.G1, self.G2, self.FOUT = F1, H, G1, G2, FOUT
        self.KF1 = F1 // P                         # 6 k-tiles
        self.KH = H // P                           # 4
        self.KG1 = G1 // P                         # 2
        self.RB1 = [0, 6, C]                       # L1 round chunk bounds
        self.SPL2 = max(1, C - 3)                  # L2 round-A src chunks
        self.NFREE = min(448, self.CH)             # front free-dim unit
        assert self.CH % self.NFREE == 0
        self.FU = self.CH // self.NFREE            # free units per chunk


CFG_FULL = dict(N=50000, E=800000)


def _to_bf16(x):
    import ml_dtypes
    return np.asarray(x, np.float32).astype(ml_dtypes.bfloat16)


def _to_f8(x):
    import ml_dtypes
    return np.asarray(x, np.float32).astype(ml_dtypes.float8_e4m3)


def _row_of_node(v, cfg):
    """Gathered-table row for global node id v (vectorized).

    Layout within a (chunk k, rank c) slice is partition-major
    (row = p*BPC + nb) so the producer can scatter a whole chunk with a
    single base+consecutive-rows indirect DMA."""
    c = v // cfg.NLOC_RAW
    s = v - c * cfg.NLOC_RAW
    k = s // cfg.CH
    off = s - k * cfg.CH
    nb = off // P
    p = off - nb * P
    return k * cfg.CHR + c * cfg.CH + p * cfg.BPC + nb


def host_prep(inputs, cfg):
    """Returns (in_maps, sched). sched is baked into the built graph and
    must be identical for every core (SPMD)."""
    x1 = np.asarray(inputs["x1"], np.float32)
    x2 = np.asarray(inputs["x2"], np.float32)
    ei = np.asarray(inputs["edge_index"])
    src = ei[0].astype(np.int64)
    dst = ei[1].astype(np.int64)
    N, E, NC = cfg.N, cfg.E, cfg.NC
    assert x1.shape[0] == N and src.shape[0] == E

    deg = np.bincount(dst, minlength=N).astype(np.float64)
    dinv = np.where(deg > 0, deg ** -0.5, 0.0).astype(np.float32)
    sdeg = np.sqrt(deg).astype(np.float32)  # 1/dinv where deg>0 else 0

    # ---- per-core edge partition by dst owner, sorted by dst block ----
    owner = dst // cfg.NLOC_RAW
    dloc = dst - owner * cfg.NLOC_RAW
    dblk = dloc // P
    drel_all = (dloc - dblk * P).astype(np.float32)
    rows_all = _row_of_node(src, cfg).astype(np.int32)

    per_core = []
    cnt = np.zeros((NC, cfg.NB), np.int64)
    for c in range(NC):
        sel = np.where(owner == c)[0]
        order = np.argsort(dblk[sel], kind="stable")
        sel = sel[order]
        b_of = dblk[sel]
        bounds = np.searchsorted(b_of, np.arange(cfg.NB + 1))
        lists = []
        for b in range(cfg.NB):
            idxs = sel[bounds[b]:bounds[b + 1]]
            lists.append((rows_all[idxs], drel_all[idxs]))
            cnt[c, b] = len(idxs)
        per_core.append(lists)

    # Two uniform cross-core layouts, each split in 2 rounds by src chunk
    # (round boundary = which AllGather chunks the gathers depend on).
    def build_layout(chunk_bounds, pad_mult):
        bounds_k = [b * cfg.CHR for b in chunk_bounds]
        layout = dict(rounds=[])
        for r in range(len(chunk_bounds) - 1):
            lo, hi = bounds_k[r], bounds_k[r + 1]
            cntr = np.zeros((NC, cfg.NB), np.int64)
            per_rc = []
            for c in range(NC):
                pc = []
                for b in range(cfg.NB):
                    rows, rel = per_core[c][b]
                    m = (rows >= lo) & (rows < hi)
                    pc.append((rows[m] - lo, rel[m]))
                    cntr[c, b] = int(m.sum())
                per_rc.append(pc)
            Kb = np.maximum(1, -(-cntr.max(axis=0) // P)).astype(np.int64)
            nb_round = int(Kb.sum())
            pad = (-nb_round) % pad_mult
            nb_round += pad
            b_of = np.concatenate([np.repeat(np.arange(cfg.NB), Kb),
                                   np.full(pad, cfg.NB - 1)])
            first = np.zeros(nb_round, bool)
            last = np.zeros(nb_round, bool)
            skip = np.ones(nb_round, bool)
            off = 0
            for b in range(cfg.NB):
                first[off] = True
                e = off + int(Kb[b])
                if b == cfg.NB - 1:
                    e = nb_round
                last[e - 1] = True
                for jj in range(int(Kb[b])):
                    skip[off + jj] = not bool((cntr[:, b] > jj * P).any())
                off += int(Kb[b])
            layout["rounds"].append(dict(Kb=Kb, nblocks=nb_round, b_of=b_of,
                                         first=first, last=last, skip=skip,
                                         per_rc=per_rc))
        return layout

    lay1 = build_layout(cfg.RB1, 16)
    lay2 = build_layout([0, cfg.SPL2, cfg.C], 32)

    def pack_layout(layout, c, JW):
        idxs, Ss = [], []
        for rr in layout["rounds"]:
            sbs = rr["nblocks"] // 8
            idx = np.zeros((sbs * P, 8), np.int32)
            drl = np.full((sbs * P, 8), -1.0, np.float32)
            g = 0
            for b in range(cfg.NB):
                rows, rel = rr["per_rc"][c][b]
                n = len(rows)
                nblk = int(rr["Kb"][b])
                if b == cfg.NB - 1:
                    nblk = rr["nblocks"] - g
                for j in range(nblk):
                    s, jj = g // 8, g % 8
                    e0 = j * P
                    m = min(P, max(0, n - e0))
                    if m > 0:
                        idx[s * P:s * P + m, jj] = rows[e0:e0 + m]
                        drl[s * P:s * P + m, jj] = rel[e0:e0 + m]
                    g += 1
            # regroup [sbs*P, 8] -> [(nblocks//JW)*P, JW] so device loads
            # are plain 2D slices (per-iteration rows)
            g8 = JW // 8
            nq = sbs // g8
            idx = (idx.reshape(nq, g8, P, 8).transpose(0, 2, 1, 3)
                   .reshape(nq * P, JW))
            drl = (drl.reshape(nq, g8, P, 8).transpose(0, 2, 1, 3)
                   .reshape(nq * P, JW))
            idxs.append(idx.copy())
            Ss.append(_to_bf16(drl))
        return (np.concatenate(idxs, axis=0), np.concatenate(Ss, axis=0))

    def mk(lay):
        return [dict(nblocks=r["nblocks"], b_of=r["b_of"], first=r["first"],
                     last=r["last"], skip=r["skip"]) for r in lay["rounds"]]
    sched = dict(lay1=mk(lay1), lay2=mk(lay2))

    # ---- weights ----
    w1 = np.asarray(inputs["mlp_w1"], np.float32)
    w2 = np.asarray(inputs["mlp_w2"], np.float32)
    w3 = np.asarray(inputs["mlp_w3"], np.float32)
    b1 = np.asarray(inputs["mlp_b1"], np.float32)
    b2 = np.asarray(inputs["mlp_b2"], np.float32)
    b3 = np.asarray(inputs["mlp_b3"], np.float32)
    W1s = np.asarray(inputs["W1"], np.float32).sum(-1)
    W12 = np.asarray(inputs["W12"], np.float32)
    W13 = np.asarray(inputs["W13"], np.float32)  # 2e-4 folded on-device
    bp1 = np.asarray(inputs["bp1"], np.float32)
    W2s = np.asarray(inputs["W2"], np.float32).sum(-1)
    bp2 = np.asarray(inputs["bp2"], np.float32)
    g1w = np.asarray(inputs["gcn1_w"], np.float32)
    g1b = np.asarray(inputs["gcn1_b"], np.float32)
    g2w = np.asarray(inputs["gcn2_w"], np.float32)
    g2b = np.asarray(inputs["gcn2_b"], np.float32)
    fcw = np.asarray(inputs["fc_w"], np.float32)
    fcb = np.asarray(inputs["fc_b"], np.float32)

    sched["bp1_nz"] = bool(np.any(bp1 != 0))
    sched["bp2_nz"] = bool(np.any(bp2 != 0))
    sched["g1b_nz"] = bool(np.any(g1b != 0))
    sched["g2b_nz"] = bool(np.any(g2b != 0))
    sched["fcb_nz"] = bool(np.any(fcb != 0))
    sched["b3_nz"] = bool(np.any(b3 != 0))
    sched["w12_ones"] = bool(np.all(W12 == 1.0))

    def pack_lhsT(w, KT, MT):
        o = np.zeros((P, KT * MT * P), np.float32)
        for k in range(KT):
            for m in range(MT):
                o[:, (k * MT + m) * P:(k * MT + m + 1) * P] = \
                    w[k * P:(k + 1) * P, m * P:(m + 1) * P]
        return o

    def pack_rhs(w, KT, F):
        o = np.zeros((P, KT * F), np.float32)
        for k in range(KT):
            o[:, k * F:(k + 1) * F] = w[k * P:(k + 1) * P, :]
        return o

    def pack_k3(w, F):
        o = np.zeros((4, F), np.float32)
        o[:3] = w
        return _to_bf16(o)

    w1_p = _to_f8(pack_lhsT(w1 * SW, cfg.KF1, cfg.KH))
    w2_p = _to_f8(pack_lhsT(w2 * SW, cfg.KH, cfg.KH))
    w3_p = _to_f8(pack_rhs(np.pad(w3 * SW, ((0, 0), (0, 1))), cfg.KH, 4))
    b1_p = b1.reshape(cfg.KH, P).T.copy()
    b2_p = b2.reshape(cfg.KH, P).T.copy()
    b3_p = np.pad(b3 * SW, (0, 1)).reshape(1, 4).repeat(P, 0).copy()
    # fused gcn1 rhs: per k-slab [g1w*16 | W13*16] -> [P, KF1*2*G1]
    gw = np.zeros((P, cfg.KF1 * 2 * cfg.G1), np.float32)
    for k in range(cfg.KF1):
        gw[:, k * 2 * cfg.G1:k * 2 * cfg.G1 + cfg.G1] = \
            g1w[k * P:(k + 1) * P, :] * SW
        gw[:, k * 2 * cfg.G1 + cfg.G1:(k + 1) * 2 * cfg.G1] = \
            W13[k * P:(k + 1) * P, :] * SW
    gw_p = _to_f8(gw)
    g2w_p = _to_bf16(pack_rhs(g2w * SW, cfg.KG1, cfg.G2))
    fcw_p = _to_bf16(fcw)
    W1s_p = pack_k3(W1s, cfg.F1)
    bp1_p = pack_k3(bp1, cfg.F1)
    W12_p = pack_k3(W12, cfg.G1)
    W2s_p = pack_k3(W2s, cfg.G1)
    bp2_p = pack_k3(bp2, cfg.G1)
    g1b_p = _to_bf16(g1b.reshape(1, cfg.G1))
    g2b_p = _to_bf16(g2b.reshape(1, cfg.G2))
    fcb_p = np.repeat(fcb.reshape(1, cfg.FOUT), P, axis=0).astype(np.float32)

    in_maps = []
    for c in range(NC):
        lo = c * cfg.NLOC_RAW
        hi = lo + cfg.NLOC_RAW
        x1T = np.zeros((cfg.F1, cfg.NLOC), np.float32)
        x1T[:, :cfg.NLOC_RAW] = x1[lo:hi].T
        x2T = np.zeros((cfg.F1, cfg.NLOC), np.float32)
        x2T[:, :cfg.NLOC_RAW] = x2[lo:hi].T

        def dpack(v):
            t = np.zeros(cfg.NLOC, np.float32)
            t[:cfg.NLOC_RAW] = v[lo:hi]
            return t.reshape(cfg.NB, P).T.copy()

        dinv4_t = dpack(dinv * (TS / SW))
        dinv16_t = dpack(dinv * SW)
        dlo_t = dpack(dinv / TS)
        sdeg_r = np.zeros((1, cfg.NLOC), np.float32)
        sdeg_r[0, :cfg.NLOC_RAW] = sdeg[lo:hi] * TS

        idx1, drel1 = pack_layout(lay1, c, 16)
        idx2, drel2 = pack_layout(lay2, c, 32)
        rows1 = (c * cfg.CH
                 + np.arange(P, dtype=np.int32)[:, None] * cfg.BPC
                 ).astype(np.int32)
        iota = np.tile(np.arange(P, dtype=np.float32), 32).reshape(1, 32 * P)
        im = {
            "identb": _to_bf16(np.eye(P, dtype=np.float32)),
            "identf": _to_f8(np.eye(P, dtype=np.float32)),
            "iota": _to_bf16(np.repeat(iota, P, axis=0)),
            "x1T": _to_f8(x1T), "x2T": _to_bf16(x2T),
            "idx1": idx1, "drel1": drel1, "idx2": idx2, "drel2": drel2,
            "rows1": rows1,
            "dinv4": dinv4_t, "dinv16": dinv16_t, "dlo": dlo_t,
            "sdeg64": _to_bf16(sdeg_r),
            "w1": w1_p, "w2": w2_p, "w3": w3_p,
            "b1": b1_p, "b2": b2_p, "b3": b3_p,
            "gw": gw_p, "g2w": g2w_p, "fcw": fcw_p,
            "W1s": W1s_p, "bp1": bp1_p, "W12": W12_p, "W2s": W2s_p,
            "bp2": bp2_p, "g1b": g1b_p, "g2b": g2b_p, "fcb": fcb_p,
        }
        in_maps.append(im)
    return in_maps, sched


def build(cfg, sched, debug=False, debug_dump=False):
    import concourse.bacc as bacc
    import concourse.bass as bass
    import concourse.mybir as mybir
    import concourse.tile as tile

    dt = mybir.dt
    AF = mybir.ActivationFunctionType
    OP = mybir.AluOpType
    AX = mybir.AxisListType

    nc = bacc.Bacc("TRN2", target_bir_lowering=False, debug=debug)

    NB, C, BPC, CH, NLOC, TR, CHR = (cfg.NB, cfg.C, cfg.BPC, cfg.CH,
                                     cfg.NLOC, cfg.TR, cfg.CHR)
    F1, H, G1, G2, FOUT = cfg.F1, cfg.H, cfg.G1, cfg.G2, cfg.FOUT
    KF1, KH, KG1 = cfg.KF1, cfg.KH, cfg.KG1
    NF, FU = cfg.NFREE, cfg.FU
    L1R = sched["lay1"]
    L2A, L2B = sched["lay2"]
    SB1 = sum(r["nblocks"] for r in L1R) // 8
    SB2T = (L2A["nblocks"] + L2B["nblocks"]) // 8
    RB1, SPL2 = cfg.RB1, cfg.SPL2
    NR1 = len(RB1) - 1

    bf = dt.bfloat16
    f8 = dt.float8e4
    f32 = dt.float32

    dd = {}

    def din(name, shape, dtype):
        dd[name] = nc.declare_dram_parameter(name, list(shape), dtype,
                                             isOutput=False)
        return dd[name]

    x1T_d = din("x1T", [F1, NLOC], f8)
    x2T_d = din("x2T", [F1, NLOC], bf)
    idx1_d = din("idx1", [SB1 // 2 * P, 16], dt.int32)
    drel1_d = din("drel1", [SB1 // 2 * P, 16], bf)
    idx2_d = din("idx2", [SB2T // 4 * P, 32], dt.int32)
    drel2_d = din("drel2", [SB2T // 4 * P, 32], bf)
    iota_d = din("iota", [P, 32 * P], bf)
    rows1_d = din("rows1", [P, 1], dt.int32)
    dinv4_d = din("dinv4", [P, NB], f32)
    dinv16_d = din("dinv16", [P, NB], f32)
    dlo_d = din("dlo", [P, NB], f32)
    sdeg_d = din("sdeg64", [1, NLOC], bf)
    w1_d = din("w1", [P, KF1 * KH * P], f8)
    w2_d = din("w2", [P, KH * KH * P], f8)
    w3_d = din("w3", [P, KH * 4], f8)
    b1_d = din("b1", [P, KH], f32)
    b2_d = din("b2", [P, KH], f32)
    b3_d = din("b3", [P, 4], f32)
    gw_d = din("gw", [P, KF1 * 2 * G1], f8)
    g2w_d = din("g2w", [P, KG1 * G2], bf)
    fcw_d = din("fcw", [G2, FOUT], bf)
    W1s_d = din("W1s", [4, F1], bf)
    bp1_d = din("bp1", [4, F1], bf)
    W12_d = din("W12", [4, G1], bf)
    W2s_d = din("W2s", [4, G1], bf)
    bp2_d = din("bp2", [4, G1], bf)
    g1b_d = din("g1b", [1, G1], bf)
    g2b_d = din("g2b", [1, G2], bf)
    fcb_d = din("fcb", [P, FOUT], f32)
    identb_d = din("identb", [P, P], bf)
    identf_d = din("identf", [P, P], f8)
    out_d = nc.declare_dram_parameter("out", [NLOC, FOUT], f32, isOutput=True)
    if debug_dump:
        dbg_gath = nc.declare_dram_parameter("dbg_gath", [P, 64 * G2], f8,
                                             isOutput=True)
        dbg_t1 = nc.declare_dram_parameter("dbg_t1", [4 * P, G1], f8,
                                           isOutput=True)
        dbg_z = nc.declare_dram_parameter("dbg_z", [P, NB * G1], bf,
                                          isOutput=True)
        dbg_sc = nc.declare_dram_parameter("dbg_sc", [P, 4 * NB], f32,
                                           isOutput=True)
        dbg_stg = nc.declare_dram_parameter("dbg_stg", [P, BPC * G1], f8,
                                            isOutput=True)
        dbg_g1a = nc.declare_dram_parameter("dbg_g1a", [4 * P, G1], f8,
                                            isOutput=True)
        dbg_gidx = nc.declare_dram_parameter("dbg_gidx", [P, 64], dt.int32,
                                             isOutput=False)
        dbg_agg2 = nc.declare_dram_parameter("dbg_agg2", [G2, NB * P], bf,
                                             isOutput=True)
        dbg_mt = nc.declare_dram_parameter("dbg_mt", [4, NLOC], bf,
                                           isOutput=True)

    with tile.TileContext(nc) as tc:
        with (
            tc.tile_pool(name="const", bufs=1) as cp,
            tc.tile_pool(name="front", bufs=2) as fp,
            tc.tile_pool(name="scat", bufs=3) as sp,
            tc.tile_pool(name="fin", bufs=2) as qp,
            tc.tile_pool(name="psG", bufs=2, space="PSUM") as psG,
            tc.tile_pool(name="psS", bufs=2, space="PSUM") as psS,
            tc.tile_pool(name="psW", bufs=2, space="PSUM") as psW,
            tc.tile_pool(name="psT", bufs=2, space="PSUM") as psT,
            tc.tile_pool(name="dram", bufs=1, space="DRAM") as dp,
        ):
            def load(dr, shape, dtype, name):
                t = cp.tile(shape, dtype, tag=name)
                nc.sync.dma_start(out=t[:, :], in_=dr[:, :])
                return t

            w1_s = load(w1_d, [P, KF1 * KH * P], f8, "w1")
            w2_s = load(w2_d, [P, KH * KH * P], f8, "w2")
            w3_s = load(w3_d, [P, KH * 4], f8, "w3")
            b1_s = load(b1_d, [P, KH], f32, "b1")
            b2_s = load(b2_d, [P, KH], f32, "b2")
            b3_s = load(b3_d, [P, 4], f32, "b3")
            gw_s = load(gw_d, [P, KF1 * 2 * G1], f8, "gw")
            g2w_s = load(g2w_d, [P, KG1 * G2], bf, "g2w")
            fcw_s = load(fcw_d, [G2, FOUT], bf, "fcw")
            W1s_s = load(W1s_d, [4, F1], bf, "W1s")
            bp1_s = load(bp1_d, [4, F1], bf, "bp1")
            W12_s = load(W12_d, [4, G1], bf, "W12")
            W2s_s = load(W2s_d, [4, G1], bf, "W2s")
            bp2_s = load(bp2_d, [4, G1], bf, "bp2")
            g1b_s = load(g1b_d, [1, G1], bf, "g1b")
            g2b_s = load(g2b_d, [1, G2], bf, "g2b")
            fcb_s = load(fcb_d, [P, FOUT], f32, "fcb")
            dinv4_s = load(dinv4_d, [P, NB], f32, "dinv4")
            dinv16_s = load(dinv16_d, [P, NB], f32, "dinv16")
            dlo_s = load(dlo_d, [P, NB], f32, "dlo")
            sdeg_s = load(sdeg_d, [1, NLOC], bf, "sdeg")

            identb = load(identb_d, [P, P], bf, "identb")
            identf = load(identf_d, [P, P], f8, "identf")
            iota_s = load(iota_d, [P, 32 * P], bf, "iota")
            rows1_s = load(rows1_d, [P, 1], dt.int32, "rows1")
            ztile = cp.tile([P, 14 * G1], f8, tag="ztile")
            nc.vector.memset(ztile[:, :], 0.0)

            mT_s = cp.tile([4, NLOC], bf, tag="mT")
            out_acc = cp.tile([P, NB * FOUT], f32, tag="oacc")
            z_s = cp.tile([P, NB * G1], bf, tag="z")
            aggA_s = cp.tile([P, NB * G1], bf, tag="aggA")
            agg2_s = cp.tile([G2, NB * P], bf, tag="agg2")

            h1t = [dp.tile([CHR, G1], f8, tag=f"h1t{k}", name=f"h1t{k}")
                   for k in range(C)]
            h2t = [dp.tile([CHR, G2], f8, tag=f"h2t{k}", name=f"h2t{k}")
                   for k in range(C)]
            h2stg = cp.tile([P, NB * G2], f8, tag="h2stg")
            # zero-fill AR input tables upfront (remote slots must be 0;
            # AllReduce(add) over disjoint slots emulates AllGather, which
            # has broken semantics in this runtime)
            ZB = CHR // P // 4
            for k in range(C):
                for z4 in range(4):
                    nc.sync.dma_start(
                        out=h1t[k][z4 * ZB * P:(z4 + 1) * ZB * P, :]
                            .rearrange("(a p) e -> p a e", p=P),
                        in_=ztile[:, :ZB * G1]
                            .rearrange("p (a e) -> p a e", e=G1))
            for k in range(C):
                for z4 in range(4):
                    nc.sync.dma_start(
                        out=h2t[k][z4 * ZB * P:(z4 + 1) * ZB * P, :]
                            .rearrange("(a p) e -> p a e", p=P),
                        in_=ztile[:, :ZB * G2]
                            .rearrange("p (a e) -> p a e", e=G2))
            h1g = [dp.tile([(RB1[r + 1] - RB1[r]) * CHR, G1], f8,
                           tag=f"h1g{r}", name=f"h1g{r}")
                   for r in range(NR1)]
            h2gA = dp.tile([SPL2 * CHR, G2], f8, tag="h2gA")
            h2gB = dp.tile([(C - SPL2) * CHR, G2], f8, tag="h2gB")

            # ================= FRONT (per chunk) =================
            for k in range(C):
                n0 = k * CH
                x1c = fp.tile([P, KF1 * CH], f8, tag="x1c")
                nc.sync.dma_start(
                    out=x1c[:, :].rearrange("p (a n) -> p a n", n=CH),
                    in_=x1T_d[:, n0:n0 + CH].rearrange("(a p) n -> p a n", p=P))
                x2c = fp.tile([P, KF1 * CH], bf, tag="x2c", bufs=1)
                nc.sync.dma_start(
                    out=x2c[:, :].rearrange("p (a n) -> p a n", n=CH),
                    in_=x2T_d[:, n0:n0 + CH].rearrange("(a p) n -> p a n", p=P))

                h1T = fp.tile([P, KH * CH], f8, tag="h1T", bufs=1)
                for u in range(FU):
                    for m in range(KH):
                        ps = psG.tile([P, NF], f32, tag="g")
                        for kk in range(KF1):
                            nc.tensor.matmul(
                                ps[:, :],
                                lhsT=w1_s[:, (kk * KH + m) * P:(kk * KH + m + 1) * P],
                                rhs=x1c[:, kk * CH + u * NF:kk * CH + u * NF + NF],
                                start=(kk == 0), stop=(kk == KF1 - 1))
                        nc.scalar.activation(
                            h1T[:, m * CH + u * NF:m * CH + u * NF + NF],
                            ps[:, :], AF.Relu, bias=b1_s[:, m:m + 1],
                            scale=1.0 / SW)
                h2T = fp.tile([P, KH * CH], f8, tag="h2T", bufs=1)
                for u in range(FU):
                    for m in range(KH):
                        ps = psG.tile([P, NF], f32, tag="g")
                        for kk in range(KH):
                            nc.tensor.matmul(
                                ps[:, :],
                                lhsT=w2_s[:, (kk * KH + m) * P:(kk * KH + m + 1) * P],
                                rhs=h1T[:, kk * CH + u * NF:kk * CH + u * NF + NF],
                                start=(kk == 0), stop=(kk == KH - 1))
                        nc.scalar.activation(
                            h2T[:, m * CH + u * NF:m * CH + u * NF + NF],
                            ps[:, :], AF.Relu, bias=b2_s[:, m:m + 1],
                            scale=1.0 / SW)

                mmc = fp.tile([P, BPC * 3], bf, tag="mmc")
                for nb in range(BPC):
                    psl = psW.tile([P, 512], f32, tag="w")
                    for kk in range(KH):
                        nc.tensor.matmul(
                            psl[:, :4],
                            lhsT=h2T[:, kk * CH + nb * P:kk * CH + (nb + 1) * P],
                            rhs=w3_s[:, kk * 4:(kk + 1) * 4],
                            start=(kk == 0), stop=(kk == KH - 1))
                    lg = fp.tile([P, 3], f32, tag="lg")
                    if sched["b3_nz"]:
                        nc.vector.tensor_add(lg[:, :], psl[:, :3], b3_s[:, :3])
                    else:
                        nc.vector.tensor_copy(lg[:, :], psl[:, :3])
                    rmax = fp.tile([P, 1], f32, tag="rmax")
                    nc.vector.reduce_max(rmax[:, :], lg[:, :], axis=AX.X)
                    mm = fp.tile([P, 3], bf, tag="mm")
                    nc.vector.tensor_scalar(
                        mm[:, :], lg[:, :], rmax[:, :1], None, OP.is_equal)
                    nc.scalar.activation(mmc[:, nb * 3:(nb + 1) * 3],
                                         mm[:, :], AF.Copy,
                                         bias=TAU_LO, scale=TAU_HI - TAU_LO)
                for nb in range(BPC):
                    b_glob = k * BPC + nb
                    pst = psT.tile([P, P], bf, tag="t")
                    nc.tensor.transpose(pst[:3, :],
                                        mmc[:, nb * 3:(nb + 1) * 3],
                                        identb[:, :])
                    nc.vector.tensor_copy(
                        mT_s[:3, b_glob * P:(b_glob + 1) * P], pst[:3, :])

                r1T = fp.tile([P, KF1 * CH], f8, tag="r1T")
                for u in range(FU):
                    for f in range(KF1):
                        psr = psG.tile([P, NF], f32, tag="g")
                        nc.tensor.matmul(
                            psr[:, :], lhsT=W1s_s[:3, f * P:(f + 1) * P],
                            rhs=mT_s[:3, n0 + u * NF:n0 + u * NF + NF],
                            start=True, stop=True)
                        if sched["bp1_nz"]:
                            psr2 = psW.tile([P, 512], f32, tag="w")
                            nc.tensor.matmul(
                                psr2[:, :NF], lhsT=bp1_s[:3, f * P:(f + 1) * P],
                                rhs=mT_s[:3, n0 + u * NF:n0 + u * NF + NF],
                                start=True, stop=True)
                            tmp = fp.tile([P, NF], f32, tag="r1tmp")
                            nc.vector.tensor_mul(
                                tmp[:, :], psr[:, :],
                                x2c[:, f * CH + u * NF:f * CH + u * NF + NF])
                            nc.vector.tensor_add(
                                r1T[:, f * CH + u * NF:f * CH + u * NF + NF],
                                tmp[:, :], psr2[:, :NF])
                        else:
                            nc.vector.tensor_mul(
                                r1T[:, f * CH + u * NF:f * CH + u * NF + NF],
                                psr[:, :],
                                x2c[:, f * CH + u * NF:f * CH + u * NF + NF])

                h1stg = fp.tile([P, BPC * G1], f8, tag="h1stg")
                for nb in range(BPC):
                    b_glob = k * BPC + nb
                    psh = psW.tile([P, 512], f32, tag="w")
                    for f in range(KF1):
                        nc.tensor.matmul(
                            psh[:, :],
                            lhsT=r1T[:, f * CH + nb * P:f * CH + (nb + 1) * P],
                            rhs=gw_s[:, f * 2 * G1:(f + 1) * 2 * G1],
                            start=(f == 0), stop=(f == KF1 - 1))
                    nc.scalar.activation(h1stg[:, nb * G1:(nb + 1) * G1],
                                         psh[:, :G1], AF.Copy,
                                         scale=dinv4_s[:, b_glob:b_glob + 1])
                    nc.scalar.activation(
                        z_s[:, b_glob * G1:(b_glob + 1) * G1],
                        psh[:, G1:2 * G1], AF.Copy, scale=2e-4 / SW)

                if debug_dump and k == C - 1:
                    nc.sync.dma_start(out=dbg_stg[:, :], in_=h1stg[:, :])
                nc.gpsimd.indirect_dma_start(
                    out=h1t[k][:, :],
                    out_offset=bass.IndirectOffsetOnAxis(
                        ap=rows1_s[:, :], axis=0),
                    in_=h1stg[:, :], in_offset=None)
                r1r = next(r for r in range(NR1)
                           if RB1[r] <= k < RB1[r + 1])
                kk0 = k - RB1[r1r]
                agt = h1g[r1r][kk0 * CHR:(kk0 + 1) * CHR, :]
                nc.gpsimd.collective_compute(
                    "AllReduce", OP.add,
                    replica_groups=[list(range(cfg.NC))],
                    ins=[h1t[k][:, :].opt()],
                    outs=[agt.opt()])

            # ================= LAYER 1 scatter (2 rounds) =================
            ps_by_b = {}

            def l1_finalize(b):
                psb = ps_by_b.pop(b)
                if sched["g1b_nz"]:
                    nc.tensor.matmul(
                        psb[:, :], lhsT=sdeg_s[:1, b * P:(b + 1) * P],
                        rhs=g1b_s[:1, :], start=False, stop=True,
                        skip_group_check=True)
                g1r = qp.tile([P, G1], bf, tag="g1r", bufs=3)
                nc.scalar.activation(g1r[:, :], psb[:, :], AF.Relu,
                                     scale=dlo_s[:, b:b + 1])
                psmw = psW.tile([P, 512], f32, tag="w")
                if not sched["w12_ones"]:
                    nc.tensor.matmul(psmw[:, :G1],
                                     lhsT=mT_s[:3, b * P:(b + 1) * P],
                                     rhs=W12_s[:3, :], start=True, stop=True)
                nc.tensor.matmul(psmw[:, G1:2 * G1],
                                 lhsT=mT_s[:3, b * P:(b + 1) * P],
                                 rhs=W2s_s[:3, :], start=True, stop=True)
                g1v = qp.tile([P, G1], bf, tag="g1v", bufs=3)
                if sched["w12_ones"]:
                    nc.vector.tensor_add(g1v[:, :], g1r[:, :],
                                         z_s[:, b * G1:(b + 1) * G1])
                else:
                    g1t = qp.tile([P, G1], bf, tag="g1t", bufs=3)
                    nc.vector.tensor_mul(g1t[:, :], g1r[:, :], psmw[:, :G1])
                    nc.vector.tensor_add(g1v[:, :], g1t[:, :],
                                         z_s[:, b * G1:(b + 1) * G1])
                r2 = qp.tile([P, G1], bf, tag="r2", bufs=3)
                if sched["bp2_nz"]:
                    psm3 = psW.tile([P, 512], f32, tag="w")
                    nc.tensor.matmul(psm3[:, :G1],
                                     lhsT=mT_s[:3, b * P:(b + 1) * P],
                                     rhs=bp2_s[:3, :], start=True, stop=True)
                    r2u = qp.tile([P, G1], f32, tag="r2u")
                    nc.vector.tensor_mul(r2u[:, :], g1v[:, :],
                                         psmw[:, G1:2 * G1])
                    r2v = qp.tile([P, G1], f32, tag="r2v")
                    nc.vector.tensor_add(r2v[:, :], r2u[:, :], psm3[:, :G1])
                    nc.vector.tensor_scalar(r2[:, :], r2v[:, :],
                                            dinv16_s[:, b:b + 1], None,
                                            OP.mult)
                else:
                    nc.vector.scalar_tensor_tensor(
                        out=r2[:, :], in0=g1v[:, :],
                        scalar=dinv16_s[:, b:b + 1],
                        in1=psmw[:, G1:2 * G1], op0=OP.mult, op1=OP.mult)
                r2T = qp.tile([P, KG1 * P], bf, tag="r2T", bufs=3)
                for f in range(KG1):
                    pst = psT.tile([P, P], bf, tag="t")
                    nc.tensor.transpose(pst[:, :], r2[:, f * P:(f + 1) * P],
                                        identb[:, :])
                    nc.vector.tensor_copy(r2T[:, f * P:(f + 1) * P],
                                          pst[:, :])
                psh2 = psW.tile([P, 512], f32, tag="w")
                for f in range(KG1):
                    nc.tensor.matmul(
                        psh2[:, :G2], lhsT=r2T[:, f * P:(f + 1) * P],
                        rhs=g2w_s[:, f * G2:(f + 1) * G2],
                        start=(f == 0), stop=(f == KG1 - 1))
                nc.scalar.activation(h2stg[:, b * G2:(b + 1) * G2],
                                     psh2[:, :G2], AF.Copy,
                                     scale=1.0 / 4.0)
                k, nb = b // BPC, b % BPC
                if nb == BPC - 1:
                    nc.gpsimd.indirect_dma_start(
                        out=h2t[k][:, :],
                        out_offset=bass.IndirectOffsetOnAxis(
                            ap=rows1_s[:, :], axis=0),
                        in_=h2stg[:, k * BPC * G2:(k + 1) * BPC * G2],
                        in_offset=None)
                    agt2 = (h2gA[k * CHR:(k + 1) * CHR, :] if k < SPL2 else
                            h2gB[(k - SPL2) * CHR:(k - SPL2 + 1) * CHR, :])
                    nc.gpsimd.collective_compute(
                        "AllReduce", OP.add,
                        replica_groups=[list(range(cfg.NC))],
                        ins=[h2t[k][:, :].opt()], outs=[agt2.opt()])

            def l1_round(meta, sb_base16, table, is_first, is_last):
                for s_loc in range(meta["nblocks"] // 16):
                    it = sb_base16 + s_loc
                    gt = sp.tile([P, 16 * G1], f8, tag="gt1", bufs=3)
                    ix = sp.tile([P, 16], dt.int32, tag="ix1")
                    nc.sync.dma_start(out=ix[:, :],
                                      in_=idx1_d[it * P:(it + 1) * P, :])
                    for jg in range(16):
                        if meta["skip"][s_loc * 16 + jg]:
                            continue
                        nc.gpsimd.indirect_dma_start(
                            out=gt[:, jg * G1:(jg + 1) * G1],
                            out_offset=None, in_=table[:, :],
                            in_offset=bass.IndirectOffsetOnAxis(
                                ap=ix[:, jg:jg + 1], axis=0))
                    dr = sp.tile([P, 16], bf, tag="dr1")
                    nc.sync.dma_start(out=dr[:, :],
                                      in_=drel1_d[it * P:(it + 1) * P, :])
                    Ssb = sp.tile([P, 16 * P], f8, tag="S1", bufs=3)
                    nc.vector.tensor_tensor(
                        out=Ssb[:, :].rearrange("p (j c) -> p j c", c=P),
                        in0=iota_s[:, :16 * P].rearrange("p (j c) -> p j c",
                                                         c=P),
                        in1=dr[:, :].unsqueeze(2).to_broadcast([P, 16, P]),
                        op=OP.is_equal)
                    for j in range(16):
                        g = s_loc * 16 + j
                        b = int(meta["b_of"][g])
                        first = bool(meta["first"][g])
                        last = bool(meta["last"][g])
                        if first:
                            psb = psS.tile([P, G1], f32, tag="agg",
                                           name="agg1")
                            ps_by_b[b] = psb
                            if not is_first:
                                nc.tensor.matmul(
                                    psb[:, :], lhsT=identb[:, :],
                                    rhs=aggA_s[:, b * G1:(b + 1) * G1],
                                    start=True, stop=False)
                        psb = ps_by_b[b]
                        stop = last and (not sched["g1b_nz"]
                                         if is_last else True)
                        nc.tensor.matmul(
                            psb[:, :], lhsT=Ssb[:, j * P:(j + 1) * P],
                            rhs=gt[:, j * G1:(j + 1) * G1],
                            start=(first and is_first), stop=stop)
                        if not last:
                            continue
                        if not is_last:
                            nc.vector.tensor_copy(
                                aggA_s[:, b * G1:(b + 1) * G1],
                                ps_by_b.pop(b)[:, :])
                        else:
                            l1_finalize(b)

            sb16 = 0
            for r in range(NR1):
                l1_round(L1R[r], sb16, h1g[r], r == 0, r == NR1 - 1)
                sb16 += L1R[r]["nblocks"] // 16

            # ================= LAYER 2 scatter (2 rounds, transposed) ======
            ps2 = {}

            def l2_finalize(b):
                psb2 = ps2.pop(b)
                if sched["g2b_nz"]:
                    nc.tensor.matmul(
                        psb2[:, :], lhsT=g2b_s[:1, :],
                        rhs=sdeg_s[:1, b * P:(b + 1) * P], start=False,
                        stop=True, skip_group_check=True)
                g2T = qp.tile([G2, P], bf, tag="g2T")
                nc.scalar.activation(g2T[:, :], psb2[:, :], AF.Relu)
                psf = psW.tile([P, 512], f32, tag="w")
                nc.tensor.matmul(psf[:, :FOUT], lhsT=g2T[:, :],
                                 rhs=fcw_s[:, :], start=True, stop=True)
                nc.scalar.activation(
                    out_acc[:, b * FOUT:(b + 1) * FOUT], psf[:, :FOUT],
                    AF.Copy, scale=dlo_s[:, b:b + 1])
                if sched["fcb_nz"]:
                    nc.vector.tensor_add(
                        out_acc[:, b * FOUT:(b + 1) * FOUT],
                        out_acc[:, b * FOUT:(b + 1) * FOUT],
                        fcb_s[:, :])

            def l2_round(meta, sb_base32, table, is_b):
                for q in range(meta["nblocks"] // 32):
                    it = sb_base32 + q
                    gt2 = sp.tile([P, 32 * G2], f8, tag="gt2", bufs=3)
                    ix2 = sp.tile([P, 32], dt.int32, tag="ix2")
                    nc.sync.dma_start(out=ix2[:, :],
                                      in_=idx2_d[it * P:(it + 1) * P, :])
                    for jg in range(32):
                        if meta["skip"][q * 32 + jg]:
                            continue
                        nc.gpsimd.indirect_dma_start(
                            out=gt2[:, jg * G2:(jg + 1) * G2],
                            out_offset=None, in_=table[:, :],
                            in_offset=bass.IndirectOffsetOnAxis(
                                ap=ix2[:, jg:jg + 1], axis=0))
                    dr2 = sp.tile([P, 32], bf, tag="dr2")
                    nc.sync.dma_start(out=dr2[:, :],
                                      in_=drel2_d[it * P:(it + 1) * P, :])
                    S2 = sp.tile([P, 32 * P], f8, tag="S2", bufs=2)
                    nc.vector.tensor_tensor(
                        out=S2[:, :].rearrange("p (j c) -> p j c", c=P),
                        in0=iota_s[:, :].rearrange("p (j c) -> p j c", c=P),
                        in1=dr2[:, :].unsqueeze(2).to_broadcast([P, 32, P]),
                        op=OP.is_equal)
                    for j in range(32):
                        g = q * 32 + j
                        b = int(meta["b_of"][g])
                        first = bool(meta["first"][g])
                        last = bool(meta["last"][g])
                        if first:
                            psb2 = psS.tile([G2, P], f32, tag="agg",
                                            name="agg2")
                            ps2[b] = psb2
                            if is_b:
                                nc.tensor.matmul(
                                    psb2[:, :], lhsT=identb[:G2, :G2],
                                    rhs=agg2_s[:, b * P:(b + 1) * P],
                                    start=True, stop=False)
                        psb2 = ps2[b]
                        stop = last and (not sched["g2b_nz"] if is_b else True)
                        nc.tensor.matmul(
                            psb2[:, :], lhsT=gt2[:, j * G2:(j + 1) * G2],
                            rhs=S2[:, j * P:(j + 1) * P],
                            start=(first and not is_b), stop=stop)
                        if not last:
                            continue
                        if not is_b:
                            nc.vector.tensor_copy(
                                agg2_s[:, b * P:(b + 1) * P],
                                ps2.pop(b)[:, :])
                        else:
                            l2_finalize(b)

            l2_round(L2A, 0, h2gA, False)
            l2_round(L2B, L2A["nblocks"] // 32, h2gB, True)

            # batched log_softmax over all node blocks (logits are tiny:
            # exp without max-shift is safe)
            e_all = qp.tile([P, NB * FOUT], f32, tag="eall", bufs=1)
            nc.scalar.activation(e_all[:, :], out_acc[:, :], AF.Exp)
            sums = qp.tile([P, NB], f32, tag="sums", bufs=1)
            nc.vector.reduce_sum(
                sums[:, :],
                e_all[:, :].rearrange("p (b f) -> p b f", f=FOUT),
                axis=AX.X)
            lns = qp.tile([P, NB], f32, tag="lns", bufs=1)
            nc.scalar.activation(lns[:, :], sums[:, :], AF.Ln)
            res = qp.tile([P, NB * FOUT], f32, tag="eall", bufs=1, name="res")
            nc.vector.tensor_tensor(
                out=res[:, :].rearrange("p (b f) -> p b f", f=FOUT),
                in0=out_acc[:, :].rearrange("p (b f) -> p b f", f=FOUT),
                in1=lns[:, :].unsqueeze(2).to_broadcast([P, NB, FOUT]),
                op=OP.subtract)
            nc.scalar.dma_start(
                out=out_d[:, :].rearrange("(b p) f -> p b f", p=P),
                in_=res[:, :].rearrange("p (b f) -> p b f", f=FOUT))
            if debug_dump:
                for (srcten, dstten, tg) in [(h1t[0], dbg_t1, "d1"),
                                             (h1g[0], dbg_g1a, "d2")]:
                    tb = sp.tile([P, 4 * G1], f8, tag=tg, bufs=1)
                    nc.sync.dma_start(
                        out=tb[:, :].rearrange("p (a e) -> p a e", e=G1),
                        in_=srcten[:4 * P, :]
                            .rearrange("(a p) e -> p a e", p=P))
                    nc.sync.dma_start(
                        out=dstten[:, :].rearrange("(a p) e -> p a e", p=P),
                        in_=tb[:, :].rearrange("p (a e) -> p a e", e=G1))
                nc.sync.dma_start(out=dbg_z[:, :], in_=z_s[:, :])
                nc.sync.dma_start(out=dbg_sc[:, :NB], in_=dinv4_s[:, :])
                nc.sync.dma_start(out=dbg_sc[:, NB:2 * NB],
                                  in_=dinv16_s[:, :])
                nc.sync.dma_start(out=dbg_sc[:, 2 * NB:3 * NB],
                                  in_=dlo_s[:, :])
                nc.sync.dma_start(
                    out=dbg_sc[:, 3 * NB:3 * NB + BPC],
                    in_=rows1_s[:, :].bitcast(f32))
                gix = sp.tile([P, 64], dt.int32, tag="gix", bufs=1)
                nc.sync.dma_start(out=gix[:, :], in_=dbg_gidx[:, :])
                ggt = sp.tile([P, 64 * G2], f8, tag="ggt", bufs=1)
                for jg in range(64):
                    nc.gpsimd.indirect_dma_start(
                        out=ggt[:, jg * G2:(jg + 1) * G2], out_offset=None,
                        in_=h2gA[:, :],
                        in_offset=bass.IndirectOffsetOnAxis(
                            ap=gix[:, jg:jg + 1], axis=0))
                nc.sync.dma_start(out=dbg_gath[:, :], in_=ggt[:, :])
                nc.sync.dma_start(out=dbg_agg2[:, :], in_=agg2_s[:, :])
                nc.sync.dma_start(out=dbg_mt[:, :], in_=mT_s[:, :])
    return nc


_LAST_EXEC_NS = None
_LAST_RESULT = None


def run(inputs, cfg, trace=False, debug=False, debug_dump=False):
    global _LAST_EXEC_NS, _LAST_RESULT
    in_maps, sched = host_prep(inputs, cfg)
    nc = build(cfg, sched, debug=debug, debug_dump=debug_dump)
    nc.finalize()
    from concourse import bass_utils
    res = bass_utils.run_bass_kernel_spmd(
        nc, in_maps, core_ids=list(range(cfg.NC)), trace=trace)
    _LAST_EXEC_NS = res.exec_time_ns
    _LAST_RESULT = res
    outs = [np.asarray(res.results[c]["out"])[:cfg.NLOC_RAW]
            for c in range(cfg.NC)]
    return np.concatenate(outs, 0).astype(np.float32)


def kernel(**inputs):
    return run(inputs, _Cfg(**CFG_FULL))
